# revision 33
# baseline (speedup 1.0000x reference)
"""Trainium2 Bass kernel for nn_Attention (dense transformer spatial attention).

Reference computation (per batch b of 4):
  X = x[b] reshaped [256, 4096]                      (4096 = 64*64 pixels)
  QKV = w_qkv @ X -> [384, 4096]; q,k,v = split(QKV) each [128, 4096]
  per head h (4 heads x 32 dims): sim = (q_h*scale)^T k_h   [4096, 4096]
  attn = softmax(sim, axis=-1); out_h = attn @ v_h^T        [4096, 32]
  H = concat_heads -> [128, 4096]; out = w_out @ H + b_out  [256, 4096]

Sharding: 8 cores = (batch b in 0..3) x (query half qh in 0..1). Each core
computes attention output for its 2048 queries over all 4096 keys plus the
final projection. The host ROTATES the key axis per core so the core's
query block is always columns 0:2048 of x (softmax is permutation-invariant
over keys), which removes the separate query-slice blob and lets the kernel
start on DMA piece 0.

Engine budget per core (TimelineSim cost model):
  - PE is the wall (~230us): sim (128 j-steps x 4 heads x 512 i rows) and
    AV (same row count) are both row-rate bound and irreducible at K<=128;
    fp8 DoubleRow was rejected on accuracy (3e-2 > gate).
  - softmax exp (262144 free-elems/core) is SPLIT between the ScalarE
    activation (Exp, ~1.04us/1024-block) and GPSIMD pow (base e^scale
    raised to the raw sim: (e^s)^x = e^(s*x), ~1.5us/block). GPSIMD cannot
    read PSUM, so Pool-assigned blocks stage sim through SBUF via a DVE
    copy (DVE has slack). This keeps ACT ~200us < PE.
  - sim is computed TRANSPOSED simT[j,i] (row-packed K=32 matmuls, one per
    head at tile_position (32h,0)) so the big attention matrix is never
    transposed; softmax max-subtraction is skipped (|scale*sim| < ~30 is
    safe in f32/bf16).
  - denominator: vT is augmented with a ones column (zero-padded to M=64
    for the (0,po) col-tiled AV matmul), so AV computes sum_j exp*v AND
    sum_j exp in one pass.
  - v is projected channel-major like k (full-rate ap=512 matmuls), then
    PE-transposed per 128-j block (f32r transpose, 1.5 c/r) into the AV
    weight layout — cheaper than the direct j-major projection whose
    ap=128 f32r matmuls run at 1/4 rate.
  - i is processed in 4 chunks of 512 so the AV PSUM accumulators are one
    bank each and double-buffer across chunks (A0/A1/B0/B1 tags): chunk
    c+1 accumulates while chunk c's epilogue drains. Phase-1 projection
    tiles and per-chunk projection psum reuse the same 4 tags.
  - epilogue: stage den rows (DVE), reciprocal (DVE, cost-model cheap),
    bounce through DRAM only for the partition-broadcast (SBUF DMA cannot
    stride-0 the source partition), normalize with DVE mults, project,
    add bias, DMA out per 128-query block.
  - phase-1 k/v/q projection sub-tiles ([128,512], one PSUM bank) and the
    32 v-transposes are interleaved into chunk 0's step stream through the
    A1/B1 tags, so the first exp fires at ~4us instead of after all of
    phase 1.
"""

import numpy as np

import concourse.bacc as bacc
import concourse.bass as bass
import concourse.mybir as mybir
import concourse.tile as tile
from concourse.bass_utils import run_bass_kernel_spmd


F32 = mybir.dt.float32
F32R = mybir.dt.float32r
BF16 = mybir.dt.bfloat16
FP8 = mybir.dt.float8e4

HEADS = 4
DH = 32                      # dim per head
C = 256                      # input channels
NJ = 4096                    # keys per batch (64*64)
NI = 2048                    # queries per core (half of 4096)
JT = 128                     # j tile (partition dim of simT)
NJT = NJ // JT               # 32 j tiles
CHUNK = 512                  # max i chunk held in AV psum accumulators
CHUNKS = [(0, 512), (512, 512), (1024, 512), (1536, 512)]
NCH = len(CHUNKS)
SCALE = float(DH) ** -0.5
BW = 3 * 128 + NJ            # blob256 width: [wqkvT (384) | x rotated (4096)]
XO = 3 * 128                 # x offset within blob256
PIECE = 512                  # x DMA piece (columns)


def _pool_heads(c, jt):
    """Schedule: which heads' exp blocks run on GPSIMD pow instead of
    ScalarE at step (chunk c, j-tile jt). Targets ~160 of 512 blocks so
    ACT busy (~215us) stays under the PE wall (~238us). Chunk 0 starts
    late (GPSIMD runs the big memsets first); chunk 3 ends early (the
    tail should not wait on a Pool chain)."""
    if c == 0:
        # GPSIMD runs the big memsets until ~9us
        if jt < 3:
            return ()
    elif jt < 3:
        # the chunk-boundary epilogue burst occupies DVE; a Pool block here
        # would wait on its DVE staging copy and stall the ex pipeline
        return ()
    if c == NCH - 1 and jt >= 26:
        return ()
    if jt % 2 == 1:
        return ((jt // 2 + c) % 4, (jt // 2 + c + 2) % 4)
    return ((jt + c) % 4,)


def build_kernel(dbg=False):
    nc = bacc.Bacc("TRN2", debug=False, num_devices=8)

    blob256_d = nc.dram_tensor("blob256", [C, BW], F32R, kind="ExternalInput").ap()
    # blob128: [woutP (256) | bias (256) | identity (128) | e4 (128) | ones row (128)]
    blob128_d = nc.dram_tensor("blob128", [128, 2 * C + 384], F32R,
                               kind="ExternalInput").ap()
    out_d = nc.dram_tensor("out_t", [NI, C], F32, kind="ExternalOutput").ap()

    with tile.TileContext(nc) as tc:
        with (
            tc.tile_pool(name="singles", bufs=1) as singles,
            tc.tile_pool(name="expp", bufs=9) as expp,
            tc.tile_pool(name="pstp", bufs=3) as pstp,
            tc.tile_pool(name="dsp", bufs=1) as dsp,
            tc.tile_pool(name="rbp", bufs=2) as rbp,
            tc.tile_pool(name="outp", bufs=6) as outp,
            tc.tile_pool(name="psim", bufs=1, space="PSUM") as psim,
            tc.tile_pool(name="pav", bufs=1, space="PSUM") as pav,
        ):
            # ---- resident SBUF tensors ----
            blob_sb = singles.tile([128, 2, BW], F32R)    # [w | x], 2 c-tiles
            w_sb = blob_sb[:, :, 0:XO]
            x_sb = blob_sb[:, :, XO:BW]
            b128_sb = singles.tile([128, 2 * C + 384], F32R)
            woutP_sb = b128_sb[:, 0:C]
            bias_sb = b128_sb[:, C:2 * C]
            id_sb = b128_sb[:, 2 * C:2 * C + 128]
            e4_sb = b128_sb[:, 2 * C + 128:2 * C + 256]
            ones1_sb = b128_sb[0:1, 2 * C + 256:2 * C + 384]
            q_sb = singles.tile([128, NI], F32R)          # rows = 4h x 32d
            k_sb = singles.tile([128, NJ], F32R)
            v_sb = singles.tile([128, NJ], F32R)          # channel-major v
            # DoubleRow AV weights: [j, jt-pair, r (j-subtile), head, 128]
            # cols 0:32 = v dims, col 32 = ones (denominator), cols 33:127
            # feed psum partitions that are never read (no zeroing needed)
            vT_sb = singles.tile([128, NJT // 2, 2, HEADS, 128], FP8)
            h_sb = singles.tile([128, NI], F32R)          # head-major rows
            ebase_sb = singles.tile([128, CHUNK], F32)    # exp(SCALE) for gpsimd pow
            bm3_sb = singles.tile([128, 1], F32)          # softmax shift (-3)

            # single SWDGE queue; pieces ordered so the kernel starts on
            # piece 0 (w + first 512 x cols, both c-tiles)
            W0 = XO + PIECE
            for ct in range(2):
                nc.sync.dma_start(out=blob_sb[:, ct, 0:W0],
                                  in_=blob256_d[ct * 128:(ct + 1) * 128, 0:W0])
            # identity/e4/ones right behind piece 0: the v-transposes need
            # the identity long before the projection needs woutP/bias
            nc.sync.dma_start(out=b128_sb[:, 2 * C:],
                              in_=blob128_d[:, 2 * C:])
            for xh in range(1, NJ // PIECE):
                lo = XO + xh * PIECE
                for ct in range(2):
                    nc.sync.dma_start(out=blob_sb[:, ct, lo:lo + PIECE],
                                      in_=blob256_d[ct * 128:(ct + 1) * 128,
                                                    lo:lo + PIECE])
            nc.sync.dma_start(out=b128_sb[:, 0:2 * C], in_=blob128_d[:, 0:2 * C])

            # trigger the ScalarE exp table load early
            warm = singles.tile([1, 1], F32)
            nc.vector.memset(warm, 0.0)
            nc.vector.memset(bm3_sb, -3.0)
            nc.scalar.activation(warm, warm, mybir.ActivationFunctionType.Exp)

            # big memsets go to GPSIMD: DVE must stay free for the phase-1
            # PSUM->SBUF copies that gate the first exp (GPSIMD is idle
            # until its first pow block at chunk 0 / jt 6)
            nc.gpsimd.memset(ebase_sb[:, :].bitcast(F32), float(np.exp(SCALE)))
            dstg0 = dsp.tile([128, CHUNK], F32, tag="dstg")
            nc.gpsimd.memset(dstg0, 1.0)
            nc.gpsimd.memset(vT_sb[:, :, :, :, DH], 1.0)   # ones column


            # ---- phase-1 projection helpers ----
            # [128, 512] PSUM sub-tiles through the pav A1/B1 tags (freed
            # fast by their DVE copy, no interaction with the sim tags that
            # pace ScalarE).
            p1_tag = [0]

            def p1_tile():
                # phase-1 tiles rotate through the sim tags (PSUM is fully
                # claimed by sim s0-s3 + av0-av3); each insert displaces one
                # sim allocation briefly, absorbed by the other heads' blocks
                t = psim.tile([128, CHUNK], F32, tag=f"s{p1_tag[0] % 4}",
                              name=f"p1_{p1_tag[0]}")
                p1_tag[0] += 1
                return t

            def emit_qkv_sub(kind, s):
                # kind: 0=q,1=k,2=v ; s: 512-col sub-tile index
                ps = p1_tile()
                for ct in range(2):
                    nc.tensor.matmul(
                        ps,
                        lhsT=w_sb[:, ct, kind * 128:(kind + 1) * 128],
                        rhs=x_sb[:, ct, s * PIECE:(s + 1) * PIECE],
                        start=(ct == 0), stop=(ct == 1),
                    )
                dst = (q_sb, k_sb, v_sb)[kind]
                nc.vector.tensor_copy(dst[:, s * PIECE:(s + 1) * PIECE], ps)

            def emit_vt(jtt):
                # transpose v block jtt: [128 vc, 128 j] -> [128 j, 128 vc],
                # then scatter (head, dim) into the DoubleRow slab layout
                ps = p1_tile()
                nc.tensor.transpose(ps[:, 0:128].bitcast(F32R),
                                    v_sb[:, jtt * JT:(jtt + 1) * JT], id_sb)
                nc.vector.tensor_copy(vT_sb[:, jtt // 2, jtt % 2, :, 0:DH],
                                      ps[:, 0:128])

            # pre-loop: enough to start chunk 0 (q/k/v over pieces 0-1,
            # vT 0-7); everything else is spread thinly over chunk 0's
            # steps, each item emitted just ahead of its first consumer so
            # a DMA-gated item never head-of-line-blocks the sim stream
            emit_qkv_sub(0, 0)
            emit_qkv_sub(1, 0)
            emit_qkv_sub(1, 1)
            emit_qkv_sub(2, 0)
            for t in range(4):
                emit_vt(t)
            emit_qkv_sub(2, 1)

            items = [
                [(1, 4), (1, 5)],              # jt0
                [(1, 6), (1, 7)],
                [(0, 1, 2)],                   # k j1024:1536
                [(0, 2, 2)],
                [(1, 8), (1, 9)],
                [(1, 10), (1, 11)],
                [(0, 1, 3)],
                [(0, 2, 3)],
                [(1, 12), (1, 13)],
                [(1, 14), (1, 15)],
                [(0, 1, 4)],
                [(0, 2, 4)],
                [(1, 16), (1, 17)],
                [(1, 18), (1, 19)],
                [(0, 1, 5)],
                [(0, 2, 5)],
                [(1, 20), (1, 21)],
                [(1, 22), (1, 23)],
                [(0, 1, 6)],
                [(0, 2, 6)],
                [(1, 24), (1, 25)],
                [(1, 26), (1, 27)],
                [(0, 1, 7)],
                [(0, 2, 7)],
                [(1, 28), (1, 29)],
                [(1, 30), (1, 31)],
                [(0, 0, 1)],                   # q i512:1024
                [(0, 0, 2)],
                [(0, 0, 3)],
            ]

            def emit_items(step):
                if step < len(items):
                    for it in items[step]:
                        if it[0] == 0:
                            emit_qkv_sub(it[1], it[2])
                        else:
                            emit_vt(it[1])

            def emit_proj(co, it, tag):
                io = co + it * 128
                pj = pav.tile([128, CHUNK], F32, tag=tag)
                pjv = pj[:, 0:C]
                nc.tensor.matmul(pjv, lhsT=h_sb[:, io:io + 128],
                                 rhs=woutP_sb, start=True, stop=False)
                # bias via ones-row outer product accumulated on the PE
                nc.tensor.matmul(pjv, lhsT=ones1_sb,
                                 rhs=bias_sb[0:1, :], start=False, stop=True)
                ot = outp.tile([128, C], F32, tag="out")
                nc.vector.tensor_copy(ot, pjv)
                nc.sync.dma_start(out=out_d[io:io + 128, :], in_=ot)

            # ---- main attention loop ----
            for c, (co, W) in enumerate(CHUNKS):
                avh = []
                for h in range(HEADS):
                    av_t = pav.tile([128, CHUNK], F32, tag=f"av{h}",
                                    name=f"av{h}_c{c}")
                    avh.append(av_t)

                def emit_av(ex, jtp):
                    # fp8 DoubleRow: contraction = 128 j partitions x 2
                    # j-subtile slabs; out = all 128 partitions of av (rows
                    # 0:32 = out dims, 32 = denominator, 33:127 junk)
                    for h in range(HEADS):
                        nc.tensor.matmul(
                            avh[h][:, 0:W],
                            lhsT=vT_sb[:, jtp, :, h, :],
                            rhs=ex[:, :, h, 0:W],
                            start=(jtp == 0), stop=(jtp == NJT // 2 - 1),
                            perf_mode=mybir.MatmulPerfMode.DoubleRow,
                        )

                pending = None
                for jtp in range(NJT // 2):
                    ex = expp.tile([128, 2, HEADS, CHUNK], FP8, tag="ex")
                    for r in range(2):
                        jt = 2 * jtp + r
                        ph = _pool_heads(c, jt)
                        for h in range(HEADS):
                            sim = psim.tile([128, CHUNK], F32, tag=f"s{h}")
                            nc.tensor.matmul(
                                sim[:, 0:W],
                                lhsT=k_sb[h * DH:(h + 1) * DH,
                                          jt * JT:(jt + 1) * JT],
                                rhs=q_sb[h * DH:(h + 1) * DH, co:co + W],
                                start=True, stop=True,
                                tile_position=(h * DH, 0),
                            )
                            exs = ex[:, r, h, 0:W]
                            if h in ph:
                                pst = pstp.tile([128, CHUNK], F32, tag="pst")
                                # shift by 3/SCALE so exp = e^(scale*sim - 3)
                                # fits e4m3 (max scaled sim ~6.8 for N(0,1)
                                # activations; e^3.8 = 45 << 448)
                                nc.vector.tensor_scalar_sub(
                                    pst[:, 0:W], sim[:, 0:W], 3.0 / SCALE)
                                nc.gpsimd.tensor_tensor(
                                    out=exs, in0=ebase_sb[:, 0:W],
                                    in1=pst[:, 0:W],
                                    op=mybir.AluOpType.pow)
                            else:
                                nc.scalar.activation(
                                    exs, sim[:, 0:W],
                                    mybir.ActivationFunctionType.Exp,
                                    scale=SCALE, bias=bm3_sb)
                        if c == 0:
                            emit_items(jt)
                    if pending is not None:
                        emit_av(*pending)
                    pending = (ex, jtp)
                emit_av(*pending)

                # ---- epilogue: softmax denominators ----
                # den rows live at psum partitions 32 & 96 of each av tile;
                # stage both rows of a tile with one stride-64 partition
                # copy, reciprocal both staged rows in one strided call,
                # ship all 4 rows to DRAM in one DMA, then 4 broadcast DMAs.
                # stage the 4 den rows (psum partitions 32/96 of avA/avB)
                # to partitions 0/32/64/96, h-major (engine partition bases
                # must be 32-aligned). Partition OFFSET shifts are legal for
                # a psum source. One full-partition reciprocal (cost is
                # free-size-bound); unused rows hold 1.0 (preset) so the e4
                # broadcast matmul contracts finite values against zeros.
                dstg = dsp.tile([128, CHUNK], F32, tag="dstg")
                rcps = dsp.tile([128, CHUNK], F32, tag="rcps")
                # ACT helps only in the last chunk (its exp stream is done);
                # mid-chunk epilogues must not block ACT's in-order queue
                cp2 = nc.scalar.copy if c == NCH - 1 else nc.vector.tensor_copy
                nc.vector.tensor_copy(dstg[0:1, 0:W], avh[0][DH:DH + 1, 0:W])
                cp2(dstg[DH:DH + 1, 0:W], avh[1][DH:DH + 1, 0:W])
                nc.vector.tensor_copy(dstg[64:65, 0:W], avh[2][DH:DH + 1, 0:W])
                cp2(dstg[96:97, 0:W], avh[3][DH:DH + 1, 0:W])
                nc.vector.reciprocal(out=rcps[:, 0:W], in_=dstg[:, 0:W])
                # partition-broadcast 1/den_h to rows h*32..h*32+31 via a
                # K=128 ones-pattern matmul (PE; e4 row 32h carries head h),
                # then copy PSUM -> SBUF
                rbP = psim.tile([128, CHUNK], F32, tag="s0")
                nc.tensor.matmul(rbP[:, 0:W], lhsT=e4_sb.bitcast(F32),
                                 rhs=rcps[:, 0:W],
                                 start=True, stop=True)
                rbC = rbp.tile([128, CHUNK], F32, tag="rb")
                nc.vector.tensor_copy(rbC[:, 0:W], rbP[:, 0:W])
                # normalize into h-major rows (psum in0 + sbuf in1 may sit at
                # different base partitions; verifier only requires equality
                # for SB+SB operand pairs)
                nits = range(W // 128) if c == NCH - 1 else (None,)
                for nit in nits:
                    fs = slice(0, W) if nit is None else slice(nit * 128,
                                                               (nit + 1) * 128)
                    for h in range(HEADS):
                        nc.vector.tensor_tensor(
                            out=h_sb[h * DH:(h + 1) * DH,
                                     co + fs.start:co + fs.stop],
                            in0=avh[h][0:DH, fs],
                            in1=rbC[h * DH:(h + 1) * DH, fs],
                            op=mybir.AluOpType.mult,
                        )
                    if nit is not None:
                        emit_proj(co, nit, f"av{nit}")
                if c < NCH - 1:
                    for it in range(W // 128):
                        emit_proj(co, it, f"av{it}")

    nc.compile()
    return nc


_NC = None


def _get_nc():
    global _NC
    if _NC is None:
        _NC = build_kernel()
    return _NC


def make_in_maps(x, w_qkv, w_out, b_out):
    x = np.ascontiguousarray(np.asarray(x, dtype=np.float32))
    w_qkv = np.asarray(w_qkv, dtype=np.float32)
    w_out = np.asarray(w_out, dtype=np.float32)
    b_out = np.asarray(b_out, dtype=np.float32)

    wqkvT = w_qkv.T                                       # [256, 384]
    # h_sb is head-major (rows h*32..h*32+31 = head h dims), so the
    # projection weight is just w_out transposed
    woutP = np.ascontiguousarray(w_out.T)                 # [128 hidden, 256]
    e4 = np.zeros((128, 128), np.float32)
    for h in range(4):
        e4[h * 32, h * 32:(h + 1) * 32] = 1.0
    ones1 = np.zeros((128, 128), np.float32)
    ones1[0, :] = 1.0
    blob128 = np.ascontiguousarray(
        np.concatenate([woutP,
                        np.broadcast_to(b_out[None, :], (128, C)),
                        np.eye(128, dtype=np.float32), e4, ones1], axis=1))

    in_maps = []
    for core in range(8):
        b, qh = divmod(core, 2)
        xb = x[b].reshape(C, NJ)
        # rotate keys so this core's queries are columns 0:NI
        xrot = np.concatenate([xb[:, qh * NI:], xb[:, :qh * NI]], axis=1)
        blob256 = np.ascontiguousarray(np.concatenate([wqkvT, xrot], axis=1))
        in_maps.append({"blob256": blob256, "blob128": blob128})
    return in_maps


def run_spmd(x, w_qkv, w_out, b_out, **kw):
    nc = _get_nc()
    in_maps = make_in_maps(x, w_qkv, w_out, b_out)
    return run_bass_kernel_spmd(nc, in_maps, core_ids=list(range(8)), **kw)


def assemble(results):
    out = np.empty((4, C, NJ), np.float32)
    for core in range(8):
        b, qh = divmod(core, 2)
        out[b, :, qh * NI:(qh + 1) * NI] = results[core]["out_t"].T
    return out.reshape(4, C, 64, 64)


def kernel(x, w_qkv, w_out, b_out):
    res = run_spmd(x, w_qkv, w_out, b_out)
    return assemble(res.results)


# revision 46
# speedup vs baseline: 1.0349x; 1.0349x over previous
"""Trainium2 Bass kernel for nn_Attention (dense transformer spatial attention).

Reference computation (per batch b of 4):
  X = x[b] reshaped [256, 4096]                      (4096 = 64*64 pixels)
  QKV = w_qkv @ X -> [384, 4096]; q,k,v = split(QKV) each [128, 4096]
  per head h (4 heads x 32 dims): sim = (q_h*scale)^T k_h   [4096, 4096]
  attn = softmax(sim, axis=-1); out_h = attn @ v_h^T        [4096, 32]
  H = concat_heads -> [128, 4096]; out = w_out @ H + b_out  [256, 4096]

Sharding: 8 cores = (batch b in 0..3) x (query half qh in 0..1). Each core
computes attention output for its 2048 queries over all 4096 keys plus the
final projection. The host ROTATES the key axis per core so the core's
query block is always columns 0:2048 of x (softmax is permutation-invariant
over keys), which removes the separate query-slice blob and lets the kernel
start on DMA piece 0.

Engine budget per core (TimelineSim cost model):
  - PE is the wall (~230us): sim (128 j-steps x 4 heads x 512 i rows) and
    AV (same row count) are both row-rate bound and irreducible at K<=128;
    fp8 DoubleRow was rejected on accuracy (3e-2 > gate).
  - softmax exp (262144 free-elems/core) is SPLIT between the ScalarE
    activation (Exp, ~1.04us/1024-block) and GPSIMD pow (base e^scale
    raised to the raw sim: (e^s)^x = e^(s*x), ~1.5us/block). GPSIMD cannot
    read PSUM, so Pool-assigned blocks stage sim through SBUF via a DVE
    copy (DVE has slack). This keeps ACT ~200us < PE.
  - sim is computed TRANSPOSED simT[j,i] (row-packed K=32 matmuls, one per
    head at tile_position (32h,0)) so the big attention matrix is never
    transposed; softmax max-subtraction is skipped (|scale*sim| < ~30 is
    safe in f32/bf16).
  - denominator: vT is augmented with a ones column (zero-padded to M=64
    for the (0,po) col-tiled AV matmul), so AV computes sum_j exp*v AND
    sum_j exp in one pass.
  - v is projected channel-major like k (full-rate ap=512 matmuls), then
    PE-transposed per 128-j block (f32r transpose, 1.5 c/r) into the AV
    weight layout — cheaper than the direct j-major projection whose
    ap=128 f32r matmuls run at 1/4 rate.
  - i is processed in 4 chunks of 512 so the AV PSUM accumulators are one
    bank each and double-buffer across chunks (A0/A1/B0/B1 tags): chunk
    c+1 accumulates while chunk c's epilogue drains. Phase-1 projection
    tiles and per-chunk projection psum reuse the same 4 tags.
  - epilogue: stage den rows (DVE), reciprocal (DVE, cost-model cheap),
    bounce through DRAM only for the partition-broadcast (SBUF DMA cannot
    stride-0 the source partition), normalize with DVE mults, project,
    add bias, DMA out per 128-query block.
  - phase-1 k/v/q projection sub-tiles ([128,512], one PSUM bank) and the
    32 v-transposes are interleaved into chunk 0's step stream through the
    A1/B1 tags, so the first exp fires at ~4us instead of after all of
    phase 1.
"""

import numpy as np

import concourse.bacc as bacc
import concourse.bass as bass
import concourse.mybir as mybir
import concourse.tile as tile
from concourse.bass_utils import run_bass_kernel_spmd


F32 = mybir.dt.float32
F32R = mybir.dt.float32r
BF16 = mybir.dt.bfloat16
FP8 = mybir.dt.float8e4

HEADS = 4
DH = 32                      # dim per head
C = 256                      # input channels
NJ = 4096                    # keys per batch (64*64)
NI = 2048                    # queries per core (half of 4096)
JT = 128                     # j tile (partition dim of simT)
NJT = NJ // JT               # 32 j tiles
CHUNK = 512                  # max i chunk held in AV psum accumulators
CHUNKS = [(0, 512), (512, 512), (1024, 512), (1536, 512)]
NCH = len(CHUNKS)
SCALE = float(DH) ** -0.5
BW = 3 * 128 + NJ            # blob256 width: [wqkvT (384) | x rotated (4096)]
XO = 3 * 128                 # x offset within blob256
PIECE = 512                  # x DMA piece (columns)


def _pool_heads(c, jt):
    """Schedule: which heads' exp blocks run on GPSIMD pow instead of
    ScalarE at step (chunk c, j-tile jt). Targets ~160 of 512 blocks so
    ACT busy (~215us) stays under the PE wall (~238us). Chunk 0 starts
    late (GPSIMD runs the big memsets first); chunk 3 ends early (the
    tail should not wait on a Pool chain)."""
    if c == 0 and jt < 3:
        return ()
    if c > 0 and jt < 3:
        return ()
    if c == NCH - 1 and jt >= 26:
        return ()
    if jt % 2 == 1:
        return ((jt // 2 + c) % 4, (jt // 2 + c + 2) % 4)
    if jt % 4 == 0:
        return ((jt // 4 + c) % 4, (jt // 4 + c + 2) % 4)
    return ((jt + c) % 4,)


def build_kernel(dbg=False):
    nc = bacc.Bacc("TRN2", debug=False, num_devices=8)

    blob256_d = nc.dram_tensor("blob256", [C, BW], F32R, kind="ExternalInput").ap()
    # blob128: [woutP (256) | bias (256) | identity (128) | e4 (128) | ones row (128)]
    blob128_d = nc.dram_tensor("blob128", [128, 2 * C + 384], F32R,
                               kind="ExternalInput").ap()
    out_d = nc.dram_tensor("out_t", [NI, C], F32, kind="ExternalOutput").ap()

    with tile.TileContext(nc) as tc:
        with (
            tc.tile_pool(name="singles", bufs=1) as singles,
            tc.tile_pool(name="expp", bufs=10) as expp,
            tc.tile_pool(name="pstp", bufs=5) as pstp,
            tc.tile_pool(name="dsp", bufs=1) as dsp,
            tc.tile_pool(name="rbp", bufs=2) as rbp,
            tc.tile_pool(name="outp", bufs=6) as outp,
            tc.tile_pool(name="psim", bufs=1, space="PSUM") as psim,
            tc.tile_pool(name="pav", bufs=1, space="PSUM") as pav,
        ):
            # ---- resident SBUF tensors ----
            blob_sb = singles.tile([128, 2, BW], F32R)    # [w | x], 2 c-tiles
            w_sb = blob_sb[:, :, 0:XO]
            x_sb = blob_sb[:, :, XO:BW]
            b128_sb = singles.tile([128, 2 * C + 384], F32R)
            woutP_sb = b128_sb[:, 0:C]
            bias_sb = b128_sb[:, C:2 * C]
            id_sb = b128_sb[:, 2 * C:2 * C + 128]
            e4_sb = b128_sb[:, 2 * C + 128:2 * C + 256]
            ones1_sb = b128_sb[0:1, 2 * C + 256:2 * C + 384]
            q_sb = singles.tile([128, NI], F32R)          # rows = 4h x 32d
            k_sb = singles.tile([128, NJ], F32R)
            v_sb = singles.tile([128, NJ], F32R)          # channel-major v
            # DoubleRow AV weights: [j, jt-pair, r (j-subtile), head, 128]
            # cols 0:32 = v dims, col 32 = ones (denominator), cols 33:127
            # feed psum partitions that are never read (no zeroing needed)
            vT_sb = singles.tile([128, NJT // 2, 2, HEADS, 128], FP8)
            h_sb = singles.tile([128, NI], F32R)          # head-major rows
            ebase_sb = singles.tile([128, CHUNK], F32)    # exp(SCALE) for gpsimd pow
            bm3_sb = singles.tile([128, 1], F32)          # softmax shift (-3)

            # single SWDGE queue; pieces ordered so the kernel starts on
            # piece 0 (w + first 512 x cols, both c-tiles)
            W0 = XO + PIECE
            for ct in range(2):
                nc.sync.dma_start(out=blob_sb[:, ct, 0:W0],
                                  in_=blob256_d[ct * 128:(ct + 1) * 128, 0:W0])
            # identity/e4/ones right behind piece 0: the v-transposes need
            # the identity long before the projection needs woutP/bias
            nc.sync.dma_start(out=b128_sb[:, 2 * C:],
                              in_=blob128_d[:, 2 * C:])
            for xh in range(1, NJ // PIECE):
                lo = XO + xh * PIECE
                for ct in range(2):
                    nc.sync.dma_start(out=blob_sb[:, ct, lo:lo + PIECE],
                                      in_=blob256_d[ct * 128:(ct + 1) * 128,
                                                    lo:lo + PIECE])
            nc.sync.dma_start(out=b128_sb[:, 0:2 * C], in_=blob128_d[:, 0:2 * C])

            # trigger the ScalarE exp table load early
            warm = singles.tile([1, 1], F32)
            nc.vector.memset(warm, 0.0)
            nc.vector.memset(bm3_sb, -3.0)
            nc.scalar.activation(warm, warm, mybir.ActivationFunctionType.Exp)

            # big memsets go to GPSIMD: DVE must stay free for the phase-1
            # PSUM->SBUF copies that gate the first exp (GPSIMD is idle
            # until its first pow block at chunk 0 / jt 6)
            nc.gpsimd.memset(ebase_sb[:, :].bitcast(F32), float(np.exp(SCALE)))
            dstg0 = dsp.tile([128, CHUNK], F32, tag="dstg")
            nc.gpsimd.memset(dstg0, 1.0)
            nc.gpsimd.memset(vT_sb[:, :, :, :, DH], 1.0)   # ones column


            # ---- phase-1 projection helpers ----
            # [128, 512] PSUM sub-tiles through the pav A1/B1 tags (freed
            # fast by their DVE copy, no interaction with the sim tags that
            # pace ScalarE).
            p1_tag = [0]

            def p1_tile():
                # phase-1 tiles rotate through the sim tags (PSUM is fully
                # claimed by sim s0-s3 + av0-av3); each insert displaces one
                # sim allocation briefly, absorbed by the other heads' blocks
                t = psim.tile([128, CHUNK], F32, tag=f"s{p1_tag[0] % 4}",
                              name=f"p1_{p1_tag[0]}")
                p1_tag[0] += 1
                return t

            def emit_qkv_sub(kind, s):
                # kind: 0=q,1=k,2=v ; s: 512-col sub-tile index
                ps = p1_tile()
                for ct in range(2):
                    nc.tensor.matmul(
                        ps,
                        lhsT=w_sb[:, ct, kind * 128:(kind + 1) * 128],
                        rhs=x_sb[:, ct, s * PIECE:(s + 1) * PIECE],
                        start=(ct == 0), stop=(ct == 1),
                    )
                dst = (q_sb, k_sb, v_sb)[kind]
                nc.vector.tensor_copy(dst[:, s * PIECE:(s + 1) * PIECE], ps)

            def emit_vt(jtt):
                # transpose v block jtt: [128 vc, 128 j] -> [128 j, 128 vc],
                # then scatter (head, dim) into the DoubleRow slab layout
                ps = p1_tile()
                nc.tensor.transpose(ps[:, 0:128].bitcast(F32R),
                                    v_sb[:, jtt * JT:(jtt + 1) * JT], id_sb)
                nc.vector.tensor_copy(vT_sb[:, jtt // 2, jtt % 2, :, 0:DH],
                                      ps[:, 0:128])

            # pre-loop: enough to start chunk 0 (q/k/v over pieces 0-1,
            # vT 0-7); everything else is spread thinly over chunk 0's
            # steps, each item emitted just ahead of its first consumer so
            # a DMA-gated item never head-of-line-blocks the sim stream
            emit_qkv_sub(0, 0)
            emit_qkv_sub(1, 0)
            emit_qkv_sub(1, 1)
            emit_qkv_sub(2, 0)
            for t in range(4):
                emit_vt(t)

            items = [
                [(0, 2, 1), (1, 4), (1, 5)],   # jt0: v j512:1024, t4-5
                [(1, 6), (1, 7)],
                [(0, 1, 2)],                   # k j1024:1536
                [(0, 2, 2)],
                [(1, 8), (1, 9)],
                [(1, 10), (1, 11)],
                [(0, 1, 3)],
                [(0, 2, 3)],
                [(1, 12), (1, 13)],
                [(1, 14), (1, 15)],
                [(0, 1, 4)],
                [(0, 2, 4)],
                [(1, 16), (1, 17)],
                [(1, 18), (1, 19)],
                [(0, 1, 5)],
                [(0, 2, 5)],
                [(1, 20), (1, 21)],
                [(1, 22), (1, 23)],
                [(0, 1, 6)],
                [(0, 2, 6)],
                [(1, 24), (1, 25)],
                [(1, 26), (1, 27)],
                [(0, 1, 7)],
                [(0, 2, 7)],
                [(1, 28), (1, 29)],
                [(1, 30), (1, 31)],
                [(0, 0, 1)],                   # q i512:1024
                [(0, 0, 2)],
                [(0, 0, 3)],
            ]

            def emit_items(step):
                if step < len(items):
                    for it in items[step]:
                        if it[0] == 0:
                            emit_qkv_sub(it[1], it[2])
                        else:
                            emit_vt(it[1])

            def emit_proj(co, it, tag, ot4=None):
                io = co + it * 128
                pj = psim.tile([128, CHUNK], F32, tag=tag,
                               name=f"pj_{co}_{it}") if tag.startswith("s") \
                    else pav.tile([128, CHUNK], F32, tag=tag,
                                  name=f"pj_{co}_{it}")
                pjv = pj[:, 0:C]
                nc.tensor.matmul(pjv, lhsT=h_sb[:, io:io + 128],
                                 rhs=woutP_sb, start=True, stop=False)
                # bias via ones-row outer product accumulated on the PE
                nc.tensor.matmul(pjv, lhsT=ones1_sb,
                                 rhs=bias_sb[0:1, :], start=False, stop=True)
                ot = outp.tile([128, C], F32, tag="out")
                if ot4 is None:
                    nc.vector.tensor_copy(ot, pjv)
                else:
                    nc.scalar.copy(ot, pjv)   # ACT is idle at the tail
                nc.sync.dma_start(out=out_d[io:io + 128, :], in_=ot)

            # ---- main attention loop ----
            for c, (co, W) in enumerate(CHUNKS):
                avh = []
                for h in range(HEADS):
                    av_t = pav.tile([128, CHUNK], F32, tag=f"av{h}",
                                    name=f"av{h}_c{c}")
                    avh.append(av_t)

                def emit_av(ex, jtp):
                    # fp8 DoubleRow: contraction = 128 j partitions x 2
                    # j-subtile slabs; out = all 128 partitions of av (rows
                    # 0:32 = out dims, 32 = denominator, 33:127 junk)
                    for h in range(HEADS):
                        nc.tensor.matmul(
                            avh[h][:, 0:W],
                            lhsT=vT_sb[:, jtp, :, h, :],
                            rhs=ex[:, :, h, 0:W],
                            start=(jtp == 0), stop=(jtp == NJT // 2 - 1),
                            perf_mode=mybir.MatmulPerfMode.DoubleRow,
                        )

                pending = None
                for jtp in range(NJT // 2):
                    ex = expp.tile([128, 2, HEADS, CHUNK], FP8, tag="ex")
                    for r in range(2):
                        jt = 2 * jtp + r
                        ph = _pool_heads(c, jt)
                        for h in range(HEADS):
                            sim = psim.tile([128, CHUNK], F32, tag=f"s{h}")
                            nc.tensor.matmul(
                                sim[:, 0:W],
                                lhsT=k_sb[h * DH:(h + 1) * DH,
                                          jt * JT:(jt + 1) * JT],
                                rhs=q_sb[h * DH:(h + 1) * DH, co:co + W],
                                start=True, stop=True,
                                tile_position=(h * DH, 0),
                            )
                            exs = ex[:, r, h, 0:W]
                            if h in ph:
                                pst = pstp.tile([128, CHUNK], F32, tag="pst")
                                # shift by 3/SCALE so exp = e^(scale*sim - 3)
                                # fits e4m3 (max scaled sim ~6.8 for N(0,1)
                                # activations; e^3.8 = 45 << 448)
                                nc.vector.tensor_scalar_sub(
                                    pst[:, 0:W], sim[:, 0:W], 3.0 / SCALE)
                                nc.gpsimd.tensor_tensor(
                                    out=exs, in0=ebase_sb[:, 0:W],
                                    in1=pst[:, 0:W],
                                    op=mybir.AluOpType.pow)
                            else:
                                nc.scalar.activation(
                                    exs, sim[:, 0:W],
                                    mybir.ActivationFunctionType.Exp,
                                    scale=SCALE, bias=bm3_sb)
                        if c == 0:
                            emit_items(jt)
                    if pending is not None:
                        emit_av(*pending)
                    pending = (ex, jtp)
                emit_av(*pending)

                # ---- epilogue: softmax denominators ----
                # den rows live at psum partitions 32 & 96 of each av tile;
                # stage both rows of a tile with one stride-64 partition
                # copy, reciprocal both staged rows in one strided call,
                # ship all 4 rows to DRAM in one DMA, then 4 broadcast DMAs.
                # stage the 4 den rows (psum partitions 32/96 of avA/avB)
                # to partitions 0/32/64/96, h-major (engine partition bases
                # must be 32-aligned). Partition OFFSET shifts are legal for
                # a psum source. One full-partition reciprocal (cost is
                # free-size-bound); unused rows hold 1.0 (preset) so the e4
                # broadcast matmul contracts finite values against zeros.
                dstg = dsp.tile([128, CHUNK], F32, tag="dstg")
                rcps = dsp.tile([128, CHUNK], F32, tag="rcps")
                # ACT helps only in the last chunk (its exp stream is done);
                # mid-chunk epilogues must not block ACT's in-order queue
                cp2 = nc.scalar.copy if c == NCH - 1 else nc.vector.tensor_copy
                nc.vector.tensor_copy(dstg[0:1, 0:W], avh[0][DH:DH + 1, 0:W])
                cp2(dstg[DH:DH + 1, 0:W], avh[1][DH:DH + 1, 0:W])
                nc.vector.tensor_copy(dstg[64:65, 0:W], avh[2][DH:DH + 1, 0:W])
                cp2(dstg[96:97, 0:W], avh[3][DH:DH + 1, 0:W])
                with nc.allow_low_precision("bf16 1/den feeds a bf16 "
                                            "broadcast matmul; 0.4% den "
                                            "error is well inside the gate"):
                    nc.vector.reciprocal(out=rcps[:, 0:W], in_=dstg[:, 0:W])
                # partition-broadcast 1/den_h to rows h*32..h*32+31 via a
                # K=128 ones-pattern matmul (PE; e4 row 32h carries head h),
                # then copy PSUM -> SBUF
                rbP = psim.tile([128, CHUNK], F32, tag="s0")
                nc.tensor.matmul(rbP[:, 0:W], lhsT=e4_sb.bitcast(F32),
                                 rhs=rcps[:, 0:W],
                                 start=True, stop=True)
                rbC = rbp.tile([128, CHUNK], F32, tag="rb")
                nc.vector.tensor_copy(rbC[:, 0:W], rbP[:, 0:W])
                # normalize into h-major rows (psum in0 + sbuf in1 may sit at
                # different base partitions; verifier only requires equality
                # for SB+SB operand pairs)
                nits = range(W // 128) if c == NCH - 1 else (None,)
                ot4 = c == NCH - 1 or None
                for nit in nits:
                    fs = slice(0, W) if nit is None else slice(nit * 128,
                                                               (nit + 1) * 128)
                    for h in range(HEADS):
                        nc.vector.tensor_tensor(
                            out=h_sb[h * DH:(h + 1) * DH,
                                     co + fs.start:co + fs.stop],
                            in0=avh[h][0:DH, fs],
                            in1=rbC[h * DH:(h + 1) * DH, fs],
                            op=mybir.AluOpType.mult,
                        )
                    if nit is not None:
                        # sim tags are idle at the tail; av tags stay locked
                        # until the last norm reads them
                        emit_proj(co, nit, f"s{1 + nit % 3}", ot4=ot4)
                if c < NCH - 1:
                    for it in range(W // 128):
                        emit_proj(co, it, f"av{it}")

    nc.compile()
    return nc


_NC = None


def _get_nc():
    global _NC
    if _NC is None:
        _NC = build_kernel()
    return _NC


def make_in_maps(x, w_qkv, w_out, b_out):
    x = np.ascontiguousarray(np.asarray(x, dtype=np.float32))
    w_qkv = np.asarray(w_qkv, dtype=np.float32)
    w_out = np.asarray(w_out, dtype=np.float32)
    b_out = np.asarray(b_out, dtype=np.float32)

    wqkvT = w_qkv.T                                       # [256, 384]
    # h_sb is head-major (rows h*32..h*32+31 = head h dims), so the
    # projection weight is just w_out transposed
    woutP = np.ascontiguousarray(w_out.T)                 # [128 hidden, 256]
    e4 = np.zeros((128, 128), np.float32)
    for h in range(4):
        e4[h * 32, h * 32:(h + 1) * 32] = 1.0
    ones1 = np.zeros((128, 128), np.float32)
    ones1[0, :] = 1.0
    blob128 = np.ascontiguousarray(
        np.concatenate([woutP,
                        np.broadcast_to(b_out[None, :], (128, C)),
                        np.eye(128, dtype=np.float32), e4, ones1], axis=1))

    in_maps = []
    for core in range(8):
        b, qh = divmod(core, 2)
        xb = x[b].reshape(C, NJ)
        # rotate keys so this core's queries are columns 0:NI
        xrot = np.concatenate([xb[:, qh * NI:], xb[:, :qh * NI]], axis=1)
        blob256 = np.ascontiguousarray(np.concatenate([wqkvT, xrot], axis=1))
        in_maps.append({"blob256": blob256, "blob128": blob128})
    return in_maps


def run_spmd(x, w_qkv, w_out, b_out, **kw):
    nc = _get_nc()
    in_maps = make_in_maps(x, w_qkv, w_out, b_out)
    return run_bass_kernel_spmd(nc, in_maps, core_ids=list(range(8)), **kw)


def assemble(results):
    out = np.empty((4, C, NJ), np.float32)
    for core in range(8):
        b, qh = divmod(core, 2)
        out[b, :, qh * NI:(qh + 1) * NI] = results[core]["out_t"].T
    return out.reshape(4, C, 64, 64)


def kernel(x, w_qkv, w_out, b_out):
    res = run_spmd(x, w_qkv, w_out, b_out)
    return assemble(res.results)


# revision 47
# speedup vs baseline: 1.0559x; 1.0203x over previous
"""Trainium2 Bass kernel for nn_Attention (dense transformer spatial attention).

Reference computation (per batch b of 4):
  X = x[b] reshaped [256, 4096]                      (4096 = 64*64 pixels)
  QKV = w_qkv @ X -> [384, 4096]; q,k,v = split(QKV) each [128, 4096]
  per head h (4 heads x 32 dims): sim = (q_h*scale)^T k_h   [4096, 4096]
  attn = softmax(sim, axis=-1); out_h = attn @ v_h^T        [4096, 32]
  H = concat_heads -> [128, 4096]; out = w_out @ H + b_out  [256, 4096]

Sharding: 8 cores = (batch b in 0..3) x (query half qh in 0..1). Each core
computes attention output for its 2048 queries over all 4096 keys plus the
final projection. The host ROTATES the key axis per core so the core's
query block is always columns 0:2048 of x (softmax is permutation-invariant
over keys), which removes the separate query-slice blob and lets the kernel
start on DMA piece 0.

Engine budget per core (TimelineSim cost model):
  - PE is the wall (~230us): sim (128 j-steps x 4 heads x 512 i rows) and
    AV (same row count) are both row-rate bound and irreducible at K<=128;
    fp8 DoubleRow was rejected on accuracy (3e-2 > gate).
  - softmax exp (262144 free-elems/core) is SPLIT between the ScalarE
    activation (Exp, ~1.04us/1024-block) and GPSIMD pow (base e^scale
    raised to the raw sim: (e^s)^x = e^(s*x), ~1.5us/block). GPSIMD cannot
    read PSUM, so Pool-assigned blocks stage sim through SBUF via a DVE
    copy (DVE has slack). This keeps ACT ~200us < PE.
  - sim is computed TRANSPOSED simT[j,i] (row-packed K=32 matmuls, one per
    head at tile_position (32h,0)) so the big attention matrix is never
    transposed; softmax max-subtraction is skipped (|scale*sim| < ~30 is
    safe in f32/bf16).
  - denominator: vT is augmented with a ones column (zero-padded to M=64
    for the (0,po) col-tiled AV matmul), so AV computes sum_j exp*v AND
    sum_j exp in one pass.
  - v is projected channel-major like k (full-rate ap=512 matmuls), then
    PE-transposed per 128-j block (f32r transpose, 1.5 c/r) into the AV
    weight layout — cheaper than the direct j-major projection whose
    ap=128 f32r matmuls run at 1/4 rate.
  - i is processed in 4 chunks of 512 so the AV PSUM accumulators are one
    bank each and double-buffer across chunks (A0/A1/B0/B1 tags): chunk
    c+1 accumulates while chunk c's epilogue drains. Phase-1 projection
    tiles and per-chunk projection psum reuse the same 4 tags.
  - epilogue: stage den rows (DVE), reciprocal (DVE, cost-model cheap),
    bounce through DRAM only for the partition-broadcast (SBUF DMA cannot
    stride-0 the source partition), normalize with DVE mults, project,
    add bias, DMA out per 128-query block.
  - phase-1 k/v/q projection sub-tiles ([128,512], one PSUM bank) and the
    32 v-transposes are interleaved into chunk 0's step stream through the
    A1/B1 tags, so the first exp fires at ~4us instead of after all of
    phase 1.
"""

import numpy as np

import concourse.bacc as bacc
import concourse.bass as bass
import concourse.mybir as mybir
import concourse.tile as tile
from concourse.bass_utils import run_bass_kernel_spmd


F32 = mybir.dt.float32
F32R = mybir.dt.float32r
BF16 = mybir.dt.bfloat16
FP8 = mybir.dt.float8e4

HEADS = 4
DH = 32                      # dim per head
C = 256                      # input channels
NJ = 4096                    # keys per batch (64*64)
NI = 2048                    # queries per core (half of 4096)
JT = 128                     # j tile (partition dim of simT)
NJT = NJ // JT               # 32 j tiles
CHUNK = 512                  # max i chunk held in AV psum accumulators
CHUNKS = [(0, 512), (512, 512), (1024, 512), (1536, 512)]
NCH = len(CHUNKS)
SCALE = float(DH) ** -0.5
BW = 3 * 128 + NJ            # blob256 width: [wqkvT (384) | x rotated (4096)]
XO = 3 * 128                 # x offset within blob256
PIECE = 512                  # x DMA piece (columns)


def _pool_heads(c, jt):
    """Schedule: which heads' exp blocks run on GPSIMD pow instead of
    ScalarE at step (chunk c, j-tile jt). Targets ~160 of 512 blocks so
    ACT busy (~215us) stays under the PE wall (~238us). Chunk 0 starts
    late (GPSIMD runs the big memsets first); chunk 3 ends early (the
    tail should not wait on a Pool chain)."""
    if c > 0 and jt < 3:
        # the chunk-boundary epilogue burst occupies DVE; a Pool block here
        # would wait on its DVE staging copy and stall the ex pipeline
        return ()
    if jt % 2 == 1:
        return ((jt // 2 + c) % 4, (jt // 2 + c + 2) % 4)
    if jt % 4 == 0:
        return ((jt // 4 + c) % 4, (jt // 4 + c + 2) % 4)
    return ((jt + c) % 4,)


def build_kernel(dbg=False):
    nc = bacc.Bacc("TRN2", debug=False, num_devices=8)

    blob256_d = nc.dram_tensor("blob256", [C, BW], F32R, kind="ExternalInput").ap()
    # blob128: [woutP (256) | bias (256) | identity (128) | e4 (128) | ones row (128)]
    blob128_d = nc.dram_tensor("blob128", [128, 2 * C + 384], F32R,
                               kind="ExternalInput").ap()
    out_d = nc.dram_tensor("out_t", [NI, C], F32, kind="ExternalOutput").ap()

    with tile.TileContext(nc) as tc:
        with (
            tc.tile_pool(name="singles", bufs=1) as singles,
            tc.tile_pool(name="expp", bufs=10) as expp,
            tc.tile_pool(name="pstp", bufs=5) as pstp,
            tc.tile_pool(name="dsp", bufs=1) as dsp,
            tc.tile_pool(name="rbp", bufs=2) as rbp,
            tc.tile_pool(name="outp", bufs=6) as outp,
            tc.tile_pool(name="psim", bufs=1, space="PSUM") as psim,
            tc.tile_pool(name="pav", bufs=1, space="PSUM") as pav,
        ):
            # ---- resident SBUF tensors ----
            blob_sb = singles.tile([128, 2, BW], F32R)    # [w | x], 2 c-tiles
            w_sb = blob_sb[:, :, 0:XO]
            x_sb = blob_sb[:, :, XO:BW]
            b128_sb = singles.tile([128, 2 * C + 384], F32R)
            woutP_sb = b128_sb[:, 0:C]
            bias_sb = b128_sb[:, C:2 * C]
            id_sb = b128_sb[:, 2 * C:2 * C + 128]
            e4_sb = b128_sb[:, 2 * C + 128:2 * C + 256]
            ones1_sb = b128_sb[0:1, 2 * C + 256:2 * C + 384]
            q_sb = singles.tile([128, NI], F32R)          # rows = 4h x 32d
            k_sb = singles.tile([128, NJ], F32R)
            v_sb = singles.tile([128, NJ], F32R)          # channel-major v
            # DoubleRow AV weights: [j, jt-pair, r (j-subtile), head, 128]
            # cols 0:32 = v dims, col 32 = ones (denominator), cols 33:127
            # feed psum partitions that are never read (no zeroing needed)
            vT_sb = singles.tile([128, NJT // 2, 2, HEADS, 128], FP8)
            h_sb = singles.tile([128, NI], F32R)          # head-major rows
            ebase_sb = singles.tile([128, CHUNK], F32)    # exp(SCALE) for gpsimd pow
            bm3_sb = singles.tile([128, 1], F32)          # softmax shift (-3)

            # single SWDGE queue; pieces ordered so the kernel starts on
            # piece 0 (w + first 512 x cols, both c-tiles)
            W0 = XO + PIECE
            for ct in range(2):
                nc.sync.dma_start(out=blob_sb[:, ct, 0:W0],
                                  in_=blob256_d[ct * 128:(ct + 1) * 128, 0:W0])
            # identity/e4/ones right behind piece 0: the v-transposes need
            # the identity long before the projection needs woutP/bias
            nc.sync.dma_start(out=b128_sb[:, 2 * C:],
                              in_=blob128_d[:, 2 * C:])
            for xh in range(1, NJ // PIECE):
                lo = XO + xh * PIECE
                for ct in range(2):
                    nc.sync.dma_start(out=blob_sb[:, ct, lo:lo + PIECE],
                                      in_=blob256_d[ct * 128:(ct + 1) * 128,
                                                    lo:lo + PIECE])
            nc.sync.dma_start(out=b128_sb[:, 0:2 * C], in_=blob128_d[:, 0:2 * C])

            # trigger the ScalarE exp table load early
            warm = singles.tile([1, 1], F32)
            nc.vector.memset(warm, 0.0)
            nc.vector.memset(bm3_sb, -3.0)
            nc.scalar.activation(warm, warm, mybir.ActivationFunctionType.Exp)

            # big memsets go to GPSIMD: DVE must stay free for the phase-1
            # PSUM->SBUF copies that gate the first exp (GPSIMD is idle
            # until its first pow block at chunk 0 / jt 6)
            nc.gpsimd.memset(ebase_sb[:, :].bitcast(F32), float(np.exp(SCALE)))
            dstg0 = dsp.tile([128, CHUNK], F32, tag="dstg")
            nc.gpsimd.memset(dstg0, 1.0)
            nc.gpsimd.memset(vT_sb[:, :, :, :, DH], 1.0)   # ones column


            # ---- phase-1 projection helpers ----
            # [128, 512] PSUM sub-tiles through the pav A1/B1 tags (freed
            # fast by their DVE copy, no interaction with the sim tags that
            # pace ScalarE).
            p1_tag = [0]

            def p1_tile():
                # phase-1 tiles rotate through the sim tags (PSUM is fully
                # claimed by sim s0-s3 + av0-av3); each insert displaces one
                # sim allocation briefly, absorbed by the other heads' blocks
                t = psim.tile([128, CHUNK], F32, tag=f"s{p1_tag[0] % 4}",
                              name=f"p1_{p1_tag[0]}")
                p1_tag[0] += 1
                return t

            def emit_qkv_sub(kind, s):
                # kind: 0=q,1=k,2=v ; s: 512-col sub-tile index
                ps = p1_tile()
                for ct in range(2):
                    nc.tensor.matmul(
                        ps,
                        lhsT=w_sb[:, ct, kind * 128:(kind + 1) * 128],
                        rhs=x_sb[:, ct, s * PIECE:(s + 1) * PIECE],
                        start=(ct == 0), stop=(ct == 1),
                    )
                dst = (q_sb, k_sb, v_sb)[kind]
                nc.vector.tensor_copy(dst[:, s * PIECE:(s + 1) * PIECE], ps)

            def emit_vt(jtt):
                # transpose v block jtt: [128 vc, 128 j] -> [128 j, 128 vc],
                # then scatter (head, dim) into the DoubleRow slab layout
                ps = p1_tile()
                nc.tensor.transpose(ps[:, 0:128].bitcast(F32R),
                                    v_sb[:, jtt * JT:(jtt + 1) * JT], id_sb)
                nc.vector.tensor_copy(vT_sb[:, jtt // 2, jtt % 2, :, 0:DH],
                                      ps[:, 0:128])

            # pre-loop: enough to start chunk 0 (q/k/v over pieces 0-1,
            # vT 0-7); everything else is spread thinly over chunk 0's
            # steps, each item emitted just ahead of its first consumer so
            # a DMA-gated item never head-of-line-blocks the sim stream
            emit_qkv_sub(0, 0)
            emit_qkv_sub(1, 0)
            emit_qkv_sub(1, 1)
            emit_qkv_sub(2, 0)
            for t in range(4):
                emit_vt(t)

            items = [
                [(0, 2, 1), (1, 4), (1, 5)],   # jt0: v j512:1024, t4-5
                [(1, 6), (1, 7)],
                [(0, 1, 2)],                   # k j1024:1536
                [(0, 2, 2)],
                [(1, 8), (1, 9)],
                [(1, 10), (1, 11)],
                [(0, 1, 3)],
                [(0, 2, 3)],
                [(1, 12), (1, 13)],
                [(1, 14), (1, 15)],
                [(0, 1, 4)],
                [(0, 2, 4)],
                [(1, 16), (1, 17)],
                [(1, 18), (1, 19)],
                [(0, 1, 5)],
                [(0, 2, 5)],
                [(1, 20), (1, 21)],
                [(1, 22), (1, 23)],
                [(0, 1, 6)],
                [(0, 2, 6)],
                [(1, 24), (1, 25)],
                [(1, 26), (1, 27)],
                [(0, 1, 7)],
                [(0, 2, 7)],
                [(1, 28), (1, 29)],
                [(1, 30), (1, 31)],
                [(0, 0, 1)],                   # q i512:1024
                [(0, 0, 2)],
                [(0, 0, 3)],
            ]

            def emit_items(step):
                if step < len(items):
                    for it in items[step]:
                        if it[0] == 0:
                            emit_qkv_sub(it[1], it[2])
                        else:
                            emit_vt(it[1])

            def emit_proj(co, it, tag, ot4=None):
                io = co + it * 128
                pj = psim.tile([128, CHUNK], F32, tag=tag,
                               name=f"pj_{co}_{it}") if tag.startswith("s") \
                    else pav.tile([128, CHUNK], F32, tag=tag,
                                  name=f"pj_{co}_{it}")
                pjv = pj[:, 0:C]
                nc.tensor.matmul(pjv, lhsT=h_sb[:, io:io + 128],
                                 rhs=woutP_sb, start=True, stop=False)
                # bias via ones-row outer product accumulated on the PE
                nc.tensor.matmul(pjv, lhsT=ones1_sb,
                                 rhs=bias_sb[0:1, :], start=False, stop=True)
                ot = outp.tile([128, C], F32, tag="out")
                if ot4 is None:
                    nc.vector.tensor_copy(ot, pjv)
                else:
                    nc.scalar.copy(ot, pjv)   # ACT is idle at the tail
                nc.sync.dma_start(out=out_d[io:io + 128, :], in_=ot)

            # ---- main attention loop ----
            for c, (co, W) in enumerate(CHUNKS):
                avh = []
                for h in range(HEADS):
                    av_t = pav.tile([128, CHUNK], F32, tag=f"av{h}",
                                    name=f"av{h}_c{c}")
                    avh.append(av_t)

                def emit_av(ex, jtp):
                    # fp8 DoubleRow: contraction = 128 j partitions x 2
                    # j-subtile slabs; out = all 128 partitions of av (rows
                    # 0:32 = out dims, 32 = denominator, 33:127 junk)
                    for h in range(HEADS):
                        nc.tensor.matmul(
                            avh[h][:, 0:W],
                            lhsT=vT_sb[:, jtp, :, h, :],
                            rhs=ex[:, :, h, 0:W],
                            start=(jtp == 0), stop=(jtp == NJT // 2 - 1),
                            perf_mode=mybir.MatmulPerfMode.DoubleRow,
                        )

                pending = None
                for jtp in range(NJT // 2):
                    ex = expp.tile([128, 2, HEADS, CHUNK], FP8, tag="ex")
                    for r in range(2):
                        jt = 2 * jtp + r
                        ph = _pool_heads(c, jt)
                        for h in range(HEADS):
                            sim = psim.tile([128, CHUNK], F32, tag=f"s{h}")
                            nc.tensor.matmul(
                                sim[:, 0:W],
                                lhsT=k_sb[h * DH:(h + 1) * DH,
                                          jt * JT:(jt + 1) * JT],
                                rhs=q_sb[h * DH:(h + 1) * DH, co:co + W],
                                start=True, stop=True,
                                tile_position=(h * DH, 0),
                            )
                            exs = ex[:, r, h, 0:W]
                            if h in ph:
                                pst = pstp.tile([128, CHUNK], F32, tag="pst")
                                # shift by 3/SCALE so exp = e^(scale*sim - 3)
                                # fits e4m3 (max scaled sim ~6.8 for N(0,1)
                                # activations; e^3.8 = 45 << 448)
                                nc.vector.tensor_scalar_sub(
                                    pst[:, 0:W], sim[:, 0:W], 3.0 / SCALE)
                                nc.gpsimd.tensor_tensor(
                                    out=exs, in0=ebase_sb[:, 0:W],
                                    in1=pst[:, 0:W],
                                    op=mybir.AluOpType.pow)
                            else:
                                nc.scalar.activation(
                                    exs, sim[:, 0:W],
                                    mybir.ActivationFunctionType.Exp,
                                    scale=SCALE, bias=bm3_sb)
                        if c == 0:
                            emit_items(jt)
                        if r == 0 and pending is not None:
                            emit_av(*pending)
                            pending = None
                    if pending is not None:
                        emit_av(*pending)
                    pending = (ex, jtp)
                emit_av(*pending)

                # ---- epilogue: softmax denominators ----
                # den rows live at psum partitions 32 & 96 of each av tile;
                # stage both rows of a tile with one stride-64 partition
                # copy, reciprocal both staged rows in one strided call,
                # ship all 4 rows to DRAM in one DMA, then 4 broadcast DMAs.
                # stage the 4 den rows (psum partitions 32/96 of avA/avB)
                # to partitions 0/32/64/96, h-major (engine partition bases
                # must be 32-aligned). Partition OFFSET shifts are legal for
                # a psum source. One full-partition reciprocal (cost is
                # free-size-bound); unused rows hold 1.0 (preset) so the e4
                # broadcast matmul contracts finite values against zeros.
                dstg = dsp.tile([128, CHUNK], F32, tag="dstg")
                rcps = dsp.tile([128, CHUNK], F32, tag="rcps")
                # ACT helps only in the last chunk (its exp stream is done);
                # mid-chunk epilogues must not block ACT's in-order queue
                cp2 = nc.scalar.copy if c == NCH - 1 else nc.vector.tensor_copy
                nc.vector.tensor_copy(dstg[0:1, 0:W], avh[0][DH:DH + 1, 0:W])
                cp2(dstg[DH:DH + 1, 0:W], avh[1][DH:DH + 1, 0:W])
                nc.vector.tensor_copy(dstg[64:65, 0:W], avh[2][DH:DH + 1, 0:W])
                cp2(dstg[96:97, 0:W], avh[3][DH:DH + 1, 0:W])
                with nc.allow_low_precision("bf16 1/den feeds a bf16 "
                                            "broadcast matmul; 0.4% den "
                                            "error is well inside the gate"):
                    nc.vector.reciprocal(out=rcps[:, 0:W], in_=dstg[:, 0:W])
                # partition-broadcast 1/den_h to rows h*32..h*32+31 via a
                # K=128 ones-pattern matmul (PE; e4 row 32h carries head h),
                # then copy PSUM -> SBUF
                rbP = psim.tile([128, CHUNK], F32, tag="s0")
                nc.tensor.matmul(rbP[:, 0:W], lhsT=e4_sb.bitcast(F32),
                                 rhs=rcps[:, 0:W],
                                 start=True, stop=True)
                rbC = rbp.tile([128, CHUNK], F32, tag="rb")
                nc.vector.tensor_copy(rbC[:, 0:W], rbP[:, 0:W])
                # normalize into h-major rows (psum in0 + sbuf in1 may sit at
                # different base partitions; verifier only requires equality
                # for SB+SB operand pairs)
                nits = range(W // 128) if c == NCH - 1 else (None,)
                ot4 = c == NCH - 1 or None
                for nit in nits:
                    fs = slice(0, W) if nit is None else slice(nit * 128,
                                                               (nit + 1) * 128)
                    for h in range(HEADS):
                        nc.vector.tensor_tensor(
                            out=h_sb[h * DH:(h + 1) * DH,
                                     co + fs.start:co + fs.stop],
                            in0=avh[h][0:DH, fs],
                            in1=rbC[h * DH:(h + 1) * DH, fs],
                            op=mybir.AluOpType.mult,
                        )
                    if nit is not None:
                        # sim tags are idle at the tail; av tags stay locked
                        # until the last norm reads them
                        emit_proj(co, nit, f"s{1 + nit % 3}", ot4=ot4)
                if c < NCH - 1:
                    for it in range(W // 128):
                        emit_proj(co, it, f"av{it}")

    nc.compile()
    return nc


_NC = None


def _get_nc():
    global _NC
    if _NC is None:
        _NC = build_kernel()
    return _NC


def make_in_maps(x, w_qkv, w_out, b_out):
    x = np.ascontiguousarray(np.asarray(x, dtype=np.float32))
    w_qkv = np.asarray(w_qkv, dtype=np.float32)
    w_out = np.asarray(w_out, dtype=np.float32)
    b_out = np.asarray(b_out, dtype=np.float32)

    wqkvT = w_qkv.T                                       # [256, 384]
    # h_sb is head-major (rows h*32..h*32+31 = head h dims), so the
    # projection weight is just w_out transposed
    woutP = np.ascontiguousarray(w_out.T)                 # [128 hidden, 256]
    e4 = np.zeros((128, 128), np.float32)
    for h in range(4):
        e4[h * 32, h * 32:(h + 1) * 32] = 1.0
    ones1 = np.zeros((128, 128), np.float32)
    ones1[0, :] = 1.0
    blob128 = np.ascontiguousarray(
        np.concatenate([woutP,
                        np.broadcast_to(b_out[None, :], (128, C)),
                        np.eye(128, dtype=np.float32), e4, ones1], axis=1))

    in_maps = []
    for core in range(8):
        b, qh = divmod(core, 2)
        xb = x[b].reshape(C, NJ)
        # rotate keys so this core's queries are columns 0:NI
        xrot = np.concatenate([xb[:, qh * NI:], xb[:, :qh * NI]], axis=1)
        blob256 = np.ascontiguousarray(np.concatenate([wqkvT, xrot], axis=1))
        in_maps.append({"blob256": blob256, "blob128": blob128})
    return in_maps


def run_spmd(x, w_qkv, w_out, b_out, **kw):
    nc = _get_nc()
    in_maps = make_in_maps(x, w_qkv, w_out, b_out)
    return run_bass_kernel_spmd(nc, in_maps, core_ids=list(range(8)), **kw)


def assemble(results):
    out = np.empty((4, C, NJ), np.float32)
    for core in range(8):
        b, qh = divmod(core, 2)
        out[b, :, qh * NI:(qh + 1) * NI] = results[core]["out_t"].T
    return out.reshape(4, C, 64, 64)


def kernel(x, w_qkv, w_out, b_out):
    res = run_spmd(x, w_qkv, w_out, b_out)
    return assemble(res.results)


# revision 57
# speedup vs baseline: 1.0720x; 1.0153x over previous
"""Trainium2 Bass kernel for nn_Attention (dense transformer spatial attention).

Reference computation (per batch b of 4):
  X = x[b] reshaped [256, 4096]                      (4096 = 64*64 pixels)
  QKV = w_qkv @ X -> [384, 4096]; q,k,v = split(QKV) each [128, 4096]
  per head h (4 heads x 32 dims): sim = (q_h*scale)^T k_h   [4096, 4096]
  attn = softmax(sim, axis=-1); out_h = attn @ v_h^T        [4096, 32]
  H = concat_heads -> [128, 4096]; out = w_out @ H + b_out  [256, 4096]

Sharding: 8 cores = (batch b in 0..3) x (query half qh in 0..1). Each core
computes attention for its 2048 queries over all 4096 keys plus the final
projection. The host ROTATES the key axis per core so the core's queries
are always columns 0:2048 of x (softmax is permutation-invariant over
keys) - no separate query blob, and the kernel starts on DMA piece 0.

Design (tuned against the TimelineSim cost model; engine busy ~us/core:
DVE 202, Pool 187, ACT 178, PE 157; wall ~259):

 - sim is computed TRANSPOSED simT[j,i] via row-packed K=32 f32r matmuls
   (one per head, tile_position (32h,0)), one [128 j, 512 i] PSUM tile per
   head (tags s0-s3, 1 bank each) so the 4-head rotation hides the
   sim->exp->sim turnaround.
 - softmax: max-subtraction is replaced by a CONSTANT shift of 3 (scaled
   sims are ~N(0,1), row maxes <= ~6.8, so e^(s*sim-3) <= ~45 fits fp8
   e4m3's 448 range; the shift cancels in the normalization). exp runs
   SPLIT across ScalarE (Exp activation, scale+bias fused) and GPSIMD
   (tensor_tensor pow: ebase^(sim - 3/s) with ebase = e^s). GPSIMD cannot
   read PSUM, so its blocks stage sim through SBUF via a DVE
   tensor_scalar_sub (which also applies the shift). ~47% of the 512
   blocks go to GPSIMD (_pool_heads), balancing ACT/DVE/Pool.
 - exp outputs are fp8 e4m3 written as two j-subtile slabs per jt-pair;
   AV then runs in fp8 DoubleRow mode: one matmul per (jt-pair, head)
   contracts 256 j in 512 moving rows (0.5 cycles/row) - 4x less PE time
   than bf16 single-row. Weights vT [128 j, pair, r, head, 128] carry the
   v dims in cols 0:32 and a ones column at 32 (denominator accumulates
   in psum row 32 for free); cols 33:127 feed junk psum rows that are
   never read. Per-head accumulators av0-av3 (1 bank each; PSUM is
   exactly sim 4 + av 4 banks).
 - accuracy: fp8 attention weights + fp8 v give rel-to-max ~1.7e-2
   (gate 2e-2); q/k stay f32r (fp8 q/k measured 3.1e-2 - rejected).
 - epilogue per 512-query chunk: stage the 4 denominator rows to
   partitions 0/32/64/96 (offset shifts are legal from a PSUM source;
   partition bases must be 32-aligned, steps of 1 only), one
   full-partition reciprocal (free-size-bound; unused rows preset to 1.0
   so nothing non-finite leaks), partition-broadcast 1/den via a K=128
   ones-pattern matmul (e4), copy to SBUF, normalize into head-major
   h_sb with DVE mults (PSUM+SBUF operands may sit at different base
   partitions), project with ONE K=128 matmul per 128 queries + a K=1
   ones-row matmul that folds the bias on the PE. The last chunk
   pipelines norm->proj per 128-query block on the idle sim tags and
   ships per-block DMAs; ScalarE (idle at the tail) does its copies.
 - phase-1: q/k/v projected in [128,512] sub-tiles, v PE-transposed per
   128-j block into the DoubleRow slab layout; all phase-1 PSUM tiles
   rotate through the sim tags, spread thinly over chunk 0's steps just
   ahead of their first consumer so DMA-gated tiles never head-of-line
   block the sim stream. Big memsets run on GPSIMD (DVE must stay free
   for the copies gating the first exp at ~9us).
 - DMA: single SWDGE queue; [w|x] blob in 512-col pieces with piece 0
   first, then identity/e4/ones constants, then remaining x, weights
   last. The DRAM reciprocal bounce of earlier designs is gone.
"""

import numpy as np

import concourse.bacc as bacc
import concourse.bass as bass
import concourse.mybir as mybir
import concourse.tile as tile
from concourse.bass_utils import run_bass_kernel_spmd


F32 = mybir.dt.float32
F32R = mybir.dt.float32r
BF16 = mybir.dt.bfloat16
FP8 = mybir.dt.float8e4

HEADS = 4
DH = 32                      # dim per head
C = 256                      # input channels
NJ = 4096                    # keys per batch (64*64)
NI = 2048                    # queries per core (half of 4096)
JT = 128                     # j tile (partition dim of simT)
NJT = NJ // JT               # 32 j tiles
CHUNK = 512                  # max i chunk held in AV psum accumulators
CHUNKS = [(0, 512), (512, 512), (1024, 512), (1536, 512)]
NCH = len(CHUNKS)
SCALE = float(DH) ** -0.5
BW = 3 * 128 + NJ            # blob256 width: [wqkvT (384) | x rotated (4096)]
XO = 3 * 128                 # x offset within blob256
PIECE = 512                  # x DMA piece (columns)


def _pool_heads(c, jt):
    """Schedule: which heads' exp blocks run on GPSIMD pow instead of
    ScalarE at step (chunk c, j-tile jt). Targets ~160 of 512 blocks so
    ACT busy (~215us) stays under the PE wall (~238us). Chunk 0 starts
    late (GPSIMD runs the big memsets first); chunk 3 ends early (the
    tail should not wait on a Pool chain)."""
    if c > 0 and jt < 3:
        # the chunk-boundary epilogue burst occupies DVE; a Pool block here
        # would wait on its DVE staging copy and stall the ex pipeline
        return ()
    if jt % 2 == 1:
        return ((jt // 2 + c) % 4, (jt // 2 + c + 2) % 4)
    if jt % 4 == 0 or (c > 0 and jt % 4 == 2):
        return ((jt // 4 + c) % 4, (jt // 4 + c + 2) % 4)
    return ((jt + c) % 4,)


def build_kernel(dbg=False):
    nc = bacc.Bacc("TRN2", debug=False, num_devices=8)

    blob256_d = nc.dram_tensor("blob256", [C, BW], F32R, kind="ExternalInput").ap()
    # blob128: [woutP (256) | bias (256) | identity (128) | e4 (128) | ones row (128)]
    blob128_d = nc.dram_tensor("blob128", [128, 2 * C + 384], F32R,
                               kind="ExternalInput").ap()
    out_d = nc.dram_tensor("out_t", [NI, C], F32, kind="ExternalOutput").ap()

    with tile.TileContext(nc) as tc:
        with (
            tc.tile_pool(name="singles", bufs=1) as singles,
            tc.tile_pool(name="expp", bufs=10) as expp,
            tc.tile_pool(name="pstp", bufs=5) as pstp,
            tc.tile_pool(name="dsp", bufs=1) as dsp,
            tc.tile_pool(name="rbp", bufs=2) as rbp,
            tc.tile_pool(name="outp", bufs=6) as outp,
            tc.tile_pool(name="psim", bufs=1, space="PSUM") as psim,
            tc.tile_pool(name="pav", bufs=1, space="PSUM") as pav,
        ):
            # ---- resident SBUF tensors ----
            blob_sb = singles.tile([128, 2, BW], F32R)    # [w | x], 2 c-tiles
            w_sb = blob_sb[:, :, 0:XO]
            x_sb = blob_sb[:, :, XO:BW]
            b128_sb = singles.tile([128, 2 * C + 384], F32R)
            woutP_sb = b128_sb[:, 0:C]
            bias_sb = b128_sb[:, C:2 * C]
            id_sb = b128_sb[:, 2 * C:2 * C + 128]
            e4_sb = b128_sb[:, 2 * C + 128:2 * C + 256]
            ones1_sb = b128_sb[0:1, 2 * C + 256:2 * C + 384]
            q_sb = singles.tile([128, NI], F32R)          # rows = 4h x 32d
            k_sb = singles.tile([128, NJ], F32R)
            v_sb = singles.tile([128, NJ], F32R)          # channel-major v
            # DoubleRow AV weights: [j, jt-pair, r (j-subtile), head, 128]
            # cols 0:32 = v dims, col 32 = ones (denominator), cols 33:127
            # feed psum partitions that are never read (no zeroing needed)
            vT_sb = singles.tile([128, NJT // 2, 2, HEADS, 128], FP8)
            h_sb = singles.tile([128, NI], F32R)          # head-major rows
            ebase_sb = singles.tile([128, CHUNK], F32)    # exp(SCALE) for gpsimd pow
            bm3_sb = singles.tile([128, 1], F32)          # softmax shift (-3)

            # single SWDGE queue; pieces ordered so the kernel starts on
            # piece 0 (w + first 512 x cols, both c-tiles)
            W0 = XO + PIECE
            for ct in range(2):
                nc.sync.dma_start(out=blob_sb[:, ct, 0:W0],
                                  in_=blob256_d[ct * 128:(ct + 1) * 128, 0:W0])
            # identity/e4/ones right behind piece 0: the v-transposes need
            # the identity long before the projection needs woutP/bias
            nc.sync.dma_start(out=b128_sb[:, 2 * C:],
                              in_=blob128_d[:, 2 * C:])
            for xh in range(1, NJ // PIECE):
                lo = XO + xh * PIECE
                for ct in range(2):
                    nc.sync.dma_start(out=blob_sb[:, ct, lo:lo + PIECE],
                                      in_=blob256_d[ct * 128:(ct + 1) * 128,
                                                    lo:lo + PIECE])
            nc.sync.dma_start(out=b128_sb[:, 0:2 * C], in_=blob128_d[:, 0:2 * C])

            # trigger the ScalarE exp table load early
            warm = singles.tile([1, 1], F32)
            nc.vector.memset(warm, 0.0)
            nc.vector.memset(bm3_sb, -3.0)
            nc.scalar.activation(warm, warm, mybir.ActivationFunctionType.Exp)

            # big memsets go to GPSIMD: DVE must stay free for the phase-1
            # PSUM->SBUF copies that gate the first exp (GPSIMD is idle
            # until its first pow block at chunk 0 / jt 6)
            nc.gpsimd.memset(ebase_sb[:, :].bitcast(F32), float(np.exp(SCALE)))
            dstg0 = dsp.tile([128, CHUNK], F32, tag="dstg")
            nc.gpsimd.memset(dstg0, 1.0)
            nc.gpsimd.memset(vT_sb[:, :, :, :, DH], 1.0)   # ones column


            # ---- phase-1 projection helpers ----
            # [128, 512] PSUM sub-tiles through the pav A1/B1 tags (freed
            # fast by their DVE copy, no interaction with the sim tags that
            # pace ScalarE).
            p1_tag = [0]

            def p1_tile():
                # phase-1 tiles rotate through the sim tags (PSUM is fully
                # claimed by sim s0-s3 + av0-av3); each insert displaces one
                # sim allocation briefly, absorbed by the other heads' blocks
                t = psim.tile([128, CHUNK], F32, tag=f"s{p1_tag[0] % 4}",
                              name=f"p1_{p1_tag[0]}")
                p1_tag[0] += 1
                return t

            def emit_qkv_sub(kind, s, on_act=False):
                # kind: 0=q,1=k,2=v ; s: 512-col sub-tile index
                ps = p1_tile()
                for ct in range(2):
                    nc.tensor.matmul(
                        ps,
                        lhsT=w_sb[:, ct, kind * 128:(kind + 1) * 128],
                        rhs=x_sb[:, ct, s * PIECE:(s + 1) * PIECE],
                        start=(ct == 0), stop=(ct == 1),
                    )
                dst = (q_sb, k_sb, v_sb)[kind]
                cp = nc.scalar.copy if on_act else nc.vector.tensor_copy
                cp(dst[:, s * PIECE:(s + 1) * PIECE], ps)

            def emit_vt(jtt):
                # transpose v block jtt: [128 vc, 128 j] -> [128 j, 128 vc],
                # then scatter (head, dim) into the DoubleRow slab layout
                ps = p1_tile()
                nc.tensor.transpose(ps[:, 0:128].bitcast(F32R),
                                    v_sb[:, jtt * JT:(jtt + 1) * JT], id_sb)
                nc.vector.tensor_copy(vT_sb[:, jtt // 2, jtt % 2, :, 0:DH],
                                      ps[:, 0:128])

            # pre-loop: enough to start chunk 0 (q/k/v over pieces 0-1,
            # vT 0-7); everything else is spread thinly over chunk 0's
            # steps, each item emitted just ahead of its first consumer so
            # a DMA-gated item never head-of-line-blocks the sim stream
            # ScalarE is idle before the first exp: let it land the q/k
            # copies the first sims are waiting on
            emit_qkv_sub(0, 0, on_act=True)
            emit_qkv_sub(1, 0, on_act=True)
            emit_qkv_sub(1, 1)
            emit_qkv_sub(2, 0)
            for t in range(4):
                emit_vt(t)

            items = [
                [(0, 2, 1), (1, 4), (1, 5)],   # jt0: v j512:1024, t4-5
                [(1, 6), (1, 7)],
                [(0, 1, 2)],                   # k j1024:1536
                [(0, 2, 2)],
                [(1, 8), (1, 9)],
                [(1, 10), (1, 11)],
                [(0, 1, 3)],
                [(0, 2, 3)],
                [(1, 12), (1, 13)],
                [(1, 14), (1, 15)],
                [(0, 1, 4)],
                [(0, 2, 4)],
                [(1, 16), (1, 17)],
                [(1, 18), (1, 19)],
                [(0, 1, 5)],
                [(0, 2, 5)],
                [(1, 20), (1, 21)],
                [(1, 22), (1, 23)],
                [(0, 1, 6)],
                [(0, 2, 6)],
                [(1, 24), (1, 25)],
                [(1, 26), (1, 27)],
                [(0, 1, 7)],
                [(0, 2, 7)],
                [(1, 28), (1, 29)],
                [(1, 30), (1, 31)],
                [(0, 0, 1)],                   # q i512:1024
                [(0, 0, 2)],
                [(0, 0, 3)],
            ]

            def emit_items(step):
                if step < len(items):
                    for it in items[step]:
                        if it[0] == 0:
                            emit_qkv_sub(it[1], it[2])
                        else:
                            emit_vt(it[1])

            def emit_proj(co, it, tag, ot4=None):
                io = co + it * 128
                pj = psim.tile([128, CHUNK], F32, tag=tag,
                               name=f"pj_{co}_{it}") if tag.startswith("s") \
                    else pav.tile([128, CHUNK], F32, tag=tag,
                                  name=f"pj_{co}_{it}")
                pjv = pj[:, 0:C]
                nc.tensor.matmul(pjv, lhsT=h_sb[:, io:io + 128],
                                 rhs=woutP_sb, start=True, stop=False)
                # bias via ones-row outer product accumulated on the PE
                nc.tensor.matmul(pjv, lhsT=ones1_sb,
                                 rhs=bias_sb[0:1, :], start=False, stop=True)
                ot = outp.tile([128, C], F32, tag="out")
                if ot4 is None:
                    nc.vector.tensor_copy(ot, pjv)
                else:
                    nc.scalar.copy(ot, pjv)   # ACT is idle at the tail
                nc.sync.dma_start(out=out_d[io:io + 128, :], in_=ot)

            # ---- main attention loop ----
            for c, (co, W) in enumerate(CHUNKS):
                avh = []
                for h in range(HEADS):
                    av_t = pav.tile([128, CHUNK], F32, tag=f"av{h}",
                                    name=f"av{h}_c{c}")
                    avh.append(av_t)

                def emit_av(ex, jtp):
                    # fp8 DoubleRow: contraction = 128 j partitions x 2
                    # j-subtile slabs; out = all 128 partitions of av (rows
                    # 0:32 = out dims, 32 = denominator, 33:127 junk)
                    for h in range(HEADS):
                        nc.tensor.matmul(
                            avh[h][:, 0:W],
                            lhsT=vT_sb[:, jtp, :, h, :],
                            rhs=ex[:, :, h, 0:W],
                            start=(jtp == 0), stop=(jtp == NJT // 2 - 1),
                            perf_mode=mybir.MatmulPerfMode.DoubleRow,
                        )

                pending = None
                for jtp in range(NJT // 2):
                    ex = expp.tile([128, 2, HEADS, CHUNK], FP8, tag="ex")
                    for r in range(2):
                        jt = 2 * jtp + r
                        ph = _pool_heads(c, jt)
                        horder = [h for h in range(HEADS) if h not in ph] + \
                            [h for h in range(HEADS) if h in ph]
                        for h in horder:
                            sim = psim.tile([128, CHUNK], F32, tag=f"s{h}")
                            nc.tensor.matmul(
                                sim[:, 0:W],
                                lhsT=k_sb[h * DH:(h + 1) * DH,
                                          jt * JT:(jt + 1) * JT],
                                rhs=q_sb[h * DH:(h + 1) * DH, co:co + W],
                                start=True, stop=True,
                                tile_position=(h * DH, 0),
                            )
                            exs = ex[:, r, h, 0:W]
                            if h in ph:
                                pst = pstp.tile([128, CHUNK], F32, tag="pst")
                                # shift by 3/SCALE so exp = e^(scale*sim - 3)
                                # fits e4m3 (max scaled sim ~6.8 for N(0,1)
                                # activations; e^3.8 = 45 << 448)
                                nc.vector.tensor_scalar_sub(
                                    pst[:, 0:W], sim[:, 0:W], 3.0 / SCALE)
                                nc.gpsimd.tensor_tensor(
                                    out=exs, in0=ebase_sb[:, 0:W],
                                    in1=pst[:, 0:W],
                                    op=mybir.AluOpType.pow)
                            else:
                                nc.scalar.activation(
                                    exs, sim[:, 0:W],
                                    mybir.ActivationFunctionType.Exp,
                                    scale=SCALE, bias=bm3_sb)
                        if c == 0:
                            emit_items(jt)
                        if r == 0 and pending is not None:
                            emit_av(*pending)
                            pending = None
                    if pending is not None:
                        emit_av(*pending)
                    pending = (ex, jtp)
                emit_av(*pending)

                # ---- epilogue: softmax denominators ----
                # den rows live at psum partitions 32 & 96 of each av tile;
                # stage both rows of a tile with one stride-64 partition
                # copy, reciprocal both staged rows in one strided call,
                # ship all 4 rows to DRAM in one DMA, then 4 broadcast DMAs.
                # stage the 4 den rows (psum partitions 32/96 of avA/avB)
                # to partitions 0/32/64/96, h-major (engine partition bases
                # must be 32-aligned). Partition OFFSET shifts are legal for
                # a psum source. One full-partition reciprocal (cost is
                # free-size-bound); unused rows hold 1.0 (preset) so the e4
                # broadcast matmul contracts finite values against zeros.
                dstg = dsp.tile([128, CHUNK], F32, tag="dstg")
                rcps = dsp.tile([128, CHUNK], F32, tag="rcps")
                # ACT helps only in the last chunk (its exp stream is done);
                # mid-chunk epilogues must not block ACT's in-order queue
                cp2 = nc.scalar.copy if c == NCH - 1 else nc.vector.tensor_copy
                nc.vector.tensor_copy(dstg[0:1, 0:W], avh[0][DH:DH + 1, 0:W])
                cp2(dstg[DH:DH + 1, 0:W], avh[1][DH:DH + 1, 0:W])
                nc.vector.tensor_copy(dstg[64:65, 0:W], avh[2][DH:DH + 1, 0:W])
                cp2(dstg[96:97, 0:W], avh[3][DH:DH + 1, 0:W])
                with nc.allow_low_precision("bf16 1/den feeds a bf16 "
                                            "broadcast matmul; 0.4% den "
                                            "error is well inside the gate"):
                    nc.vector.reciprocal(out=rcps[:, 0:W], in_=dstg[:, 0:W])
                # partition-broadcast 1/den_h to rows h*32..h*32+31 via a
                # K=128 ones-pattern matmul (PE; e4 row 32h carries head h),
                # then copy PSUM -> SBUF
                rbP = psim.tile([128, CHUNK], F32, tag="s0")
                nc.tensor.matmul(rbP[:, 0:W], lhsT=e4_sb.bitcast(F32),
                                 rhs=rcps[:, 0:W],
                                 start=True, stop=True)
                rbC = rbp.tile([128, CHUNK], F32, tag="rb")
                nc.vector.tensor_copy(rbC[:, 0:W], rbP[:, 0:W])
                # normalize into h-major rows (psum in0 + sbuf in1 may sit at
                # different base partitions; verifier only requires equality
                # for SB+SB operand pairs)
                nits = range(W // 128) if c == NCH - 1 else (None,)
                ot4 = c == NCH - 1 or None
                for nit in nits:
                    fs = slice(0, W) if nit is None else slice(nit * 128,
                                                               (nit + 1) * 128)
                    for h in range(HEADS):
                        nc.vector.tensor_tensor(
                            out=h_sb[h * DH:(h + 1) * DH,
                                     co + fs.start:co + fs.stop],
                            in0=avh[h][0:DH, fs],
                            in1=rbC[h * DH:(h + 1) * DH, fs],
                            op=mybir.AluOpType.mult,
                        )
                    if nit is not None:
                        # sim tags are idle at the tail; av tags stay locked
                        # until the last norm reads them
                        emit_proj(co, nit, f"s{1 + nit % 3}", ot4=ot4)
                if c < NCH - 1:
                    for it in range(W // 128):
                        emit_proj(co, it, f"av{it}")

    nc.compile()
    return nc


_NC = None


def _get_nc():
    global _NC
    if _NC is None:
        _NC = build_kernel()
    return _NC


def make_in_maps(x, w_qkv, w_out, b_out):
    x = np.ascontiguousarray(np.asarray(x, dtype=np.float32))
    w_qkv = np.asarray(w_qkv, dtype=np.float32)
    w_out = np.asarray(w_out, dtype=np.float32)
    b_out = np.asarray(b_out, dtype=np.float32)

    wqkvT = w_qkv.T                                       # [256, 384]
    # h_sb is head-major (rows h*32..h*32+31 = head h dims), so the
    # projection weight is just w_out transposed
    woutP = np.ascontiguousarray(w_out.T)                 # [128 hidden, 256]
    e4 = np.zeros((128, 128), np.float32)
    for h in range(4):
        e4[h * 32, h * 32:(h + 1) * 32] = 1.0
    ones1 = np.zeros((128, 128), np.float32)
    ones1[0, :] = 1.0
    blob128 = np.ascontiguousarray(
        np.concatenate([woutP,
                        np.broadcast_to(b_out[None, :], (128, C)),
                        np.eye(128, dtype=np.float32), e4, ones1], axis=1))

    in_maps = []
    for core in range(8):
        b, qh = divmod(core, 2)
        xb = x[b].reshape(C, NJ)
        # rotate keys so this core's queries are columns 0:NI
        xrot = np.concatenate([xb[:, qh * NI:], xb[:, :qh * NI]], axis=1)
        blob256 = np.ascontiguousarray(np.concatenate([wqkvT, xrot], axis=1))
        in_maps.append({"blob256": blob256, "blob128": blob128})
    return in_maps


def run_spmd(x, w_qkv, w_out, b_out, **kw):
    nc = _get_nc()
    in_maps = make_in_maps(x, w_qkv, w_out, b_out)
    return run_bass_kernel_spmd(nc, in_maps, core_ids=list(range(8)), **kw)


def assemble(results):
    out = np.empty((4, C, NJ), np.float32)
    for core in range(8):
        b, qh = divmod(core, 2)
        out[b, :, qh * NI:(qh + 1) * NI] = results[core]["out_t"].T
    return out.reshape(4, C, 64, 64)


def kernel(x, w_qkv, w_out, b_out):
    res = run_spmd(x, w_qkv, w_out, b_out)
    return assemble(res.results)


# revision 66
# speedup vs baseline: 1.0817x; 1.0090x over previous
"""Trainium2 Bass kernel for nn_Attention (dense transformer spatial attention).

Reference computation (per batch b of 4):
  X = x[b] reshaped [256, 4096]                      (4096 = 64*64 pixels)
  QKV = w_qkv @ X -> [384, 4096]; q,k,v = split(QKV) each [128, 4096]
  per head h (4 heads x 32 dims): sim = (q_h*scale)^T k_h   [4096, 4096]
  attn = softmax(sim, axis=-1); out_h = attn @ v_h^T        [4096, 32]
  H = concat_heads -> [128, 4096]; out = w_out @ H + b_out  [256, 4096]

Sharding: 8 cores = (batch b in 0..3) x (query half qh in 0..1). Each core
computes attention for its 2048 queries over all 4096 keys plus the final
projection. The host ROTATES the key axis per core so the core's queries
are always columns 0:2048 of x (softmax is permutation-invariant over
keys) - no separate query blob, and the kernel starts on DMA piece 0.

Design (tuned against the TimelineSim cost model; engine busy ~us/core:
DVE 202, Pool 187, ACT 178, PE 157; wall ~259):

 - sim is computed TRANSPOSED simT[j,i] via row-packed K=32 f32r matmuls
   (one per head, tile_position (32h,0)), one [128 j, 512 i] PSUM tile per
   head (tags s0-s3, 1 bank each) so the 4-head rotation hides the
   sim->exp->sim turnaround.
 - softmax: max-subtraction is replaced by a CONSTANT shift of 3 (scaled
   sims are ~N(0,1), row maxes <= ~6.8, so e^(s*sim-3) <= ~45 fits fp8
   e4m3's 448 range; the shift cancels in the normalization). exp runs
   SPLIT across ScalarE (Exp activation, scale+bias fused) and GPSIMD
   (tensor_tensor pow: ebase^(sim - 3/s) with ebase = e^s). GPSIMD cannot
   read PSUM, so its blocks stage sim through SBUF via a DVE
   tensor_scalar_sub (which also applies the shift). ~47% of the 512
   blocks go to GPSIMD (_pool_heads), balancing ACT/DVE/Pool.
 - exp outputs are fp8 e4m3 written as two j-subtile slabs per jt-pair;
   AV then runs in fp8 DoubleRow mode: one matmul per (jt-pair, head)
   contracts 256 j in 512 moving rows (0.5 cycles/row) - 4x less PE time
   than bf16 single-row. Weights vT [128 j, pair, r, head, 128] carry the
   v dims in cols 0:32 and a ones column at 32 (denominator accumulates
   in psum row 32 for free); cols 33:127 feed junk psum rows that are
   never read. Per-head accumulators av0-av3 (1 bank each; PSUM is
   exactly sim 4 + av 4 banks).
 - accuracy: fp8 attention weights + fp8 v give rel-to-max ~1.7e-2
   (gate 2e-2); q/k stay f32r (fp8 q/k measured 3.1e-2 - rejected).
 - epilogue per 512-query chunk: stage the 4 denominator rows to
   partitions 0/32/64/96 (offset shifts are legal from a PSUM source;
   partition bases must be 32-aligned, steps of 1 only), one
   full-partition reciprocal (free-size-bound; unused rows preset to 1.0
   so nothing non-finite leaks), partition-broadcast 1/den via a K=128
   ones-pattern matmul (e4), copy to SBUF, normalize into head-major
   h_sb with DVE mults (PSUM+SBUF operands may sit at different base
   partitions), project with ONE K=128 matmul per 128 queries + a K=1
   ones-row matmul that folds the bias on the PE. The last chunk
   pipelines norm->proj per 128-query block on the idle sim tags and
   ships per-block DMAs; ScalarE (idle at the tail) does its copies.
 - phase-1: q/k/v projected in [128,512] sub-tiles, v PE-transposed per
   128-j block into the DoubleRow slab layout; all phase-1 PSUM tiles
   rotate through the sim tags, spread thinly over chunk 0's steps just
   ahead of their first consumer so DMA-gated tiles never head-of-line
   block the sim stream. Big memsets run on GPSIMD (DVE must stay free
   for the copies gating the first exp at ~9us).
 - DMA: single SWDGE queue; [w|x] blob in 512-col pieces with piece 0
   first, then identity/e4/ones constants, then remaining x, weights
   last. The DRAM reciprocal bounce of earlier designs is gone.
"""

import numpy as np

import concourse.bacc as bacc
import concourse.bass as bass
import concourse.mybir as mybir
import concourse.tile as tile
from concourse.bass_utils import run_bass_kernel_spmd


F32 = mybir.dt.float32
F32R = mybir.dt.float32r
BF16 = mybir.dt.bfloat16
FP8 = mybir.dt.float8e4

HEADS = 4
DH = 32                      # dim per head
C = 256                      # input channels
NJ = 4096                    # keys per batch (64*64)
NI = 2048                    # queries per core (half of 4096)
JT = 128                     # j tile (partition dim of simT)
NJT = NJ // JT               # 32 j tiles
CHUNK = 512                  # max i chunk held in AV psum accumulators
CHUNKS = [(0, 512), (512, 512), (1024, 512), (1536, 512)]
NCH = len(CHUNKS)
SCALE = float(DH) ** -0.5
BW = 3 * 128 + NJ            # blob256 width: [wqkvT (384) | x rotated (4096)]
XO = 3 * 128                 # x offset within blob256
PIECE = 512                  # x DMA piece (columns)


def _pool_heads(c, jt):
    """Schedule: which heads' exp blocks run on GPSIMD pow instead of
    ScalarE at step (chunk c, j-tile jt). Targets ~160 of 512 blocks so
    ACT busy (~215us) stays under the PE wall (~238us). Chunk 0 starts
    late (GPSIMD runs the big memsets first); chunk 3 ends early (the
    tail should not wait on a Pool chain)."""
    if c > 0 and jt < 3:
        # the chunk-boundary epilogue burst occupies DVE; a Pool block here
        # would wait on its DVE staging copy and stall the ex pipeline
        return ()
    if jt % 2 == 1:
        return ((jt // 2 + c) % 4, (jt // 2 + c + 2) % 4)
    if jt % 4 == 0 or (c > 0 and jt % 4 == 2):
        return ((jt // 4 + c) % 4, (jt // 4 + c + 2) % 4)
    return ((jt + c) % 4,)


def build_kernel(dbg=False):
    nc = bacc.Bacc("TRN2", debug=False, num_devices=8)

    blob256_d = nc.dram_tensor("blob256", [C, BW], F32R, kind="ExternalInput").ap()
    # blob128: [woutP (256) | bias (256) | identity (128) | e4 (128) | ones row (128)]
    blob128_d = nc.dram_tensor("blob128", [128, 2 * C + 384], F32R,
                               kind="ExternalInput").ap()
    out_d = nc.dram_tensor("out_t", [NI, C], F32, kind="ExternalOutput").ap()

    with tile.TileContext(nc) as tc:
        with (
            tc.tile_pool(name="singles", bufs=1) as singles,
            tc.tile_pool(name="expp", bufs=10) as expp,
            tc.tile_pool(name="pstp", bufs=5) as pstp,
            tc.tile_pool(name="dsp", bufs=1) as dsp,
            tc.tile_pool(name="rbp", bufs=2) as rbp,
            tc.tile_pool(name="outp", bufs=6) as outp,
            tc.tile_pool(name="psim", bufs=1, space="PSUM") as psim,
            tc.tile_pool(name="pav", bufs=1, space="PSUM") as pav,
        ):
            # ---- resident SBUF tensors ----
            blob_sb = singles.tile([128, 2, BW], F32R)    # [w | x], 2 c-tiles
            w_sb = blob_sb[:, :, 0:XO]
            x_sb = blob_sb[:, :, XO:BW]
            b128_sb = singles.tile([128, 2 * C + 384], F32R)
            woutP_sb = b128_sb[:, 0:C]
            bias_sb = b128_sb[:, C:2 * C]
            id_sb = b128_sb[:, 2 * C:2 * C + 128]
            e4_sb = b128_sb[:, 2 * C + 128:2 * C + 256]
            ones1_sb = b128_sb[0:1, 2 * C + 256:2 * C + 384]
            q_sb = singles.tile([128, NI], F32R)          # rows = 4h x 32d
            k_sb = singles.tile([128, NJ], F32R)
            v_sb = singles.tile([128, NJ], F32R)          # channel-major v
            # DoubleRow AV weights: [j, jt-pair, r (j-subtile), head, 128]
            # cols 0:32 = v dims, col 32 = ones (denominator), cols 33:127
            # feed psum partitions that are never read (no zeroing needed)
            vT_sb = singles.tile([128, NJT // 2, 2, HEADS, 128], FP8)
            h_sb = singles.tile([128, NI], F32R)          # head-major rows
            ebase_sb = singles.tile([128, CHUNK], F32)    # exp(SCALE) for gpsimd pow
            bm3_sb = singles.tile([128, 1], F32)          # softmax shift (-3)

            # single SWDGE queue; pieces ordered so the kernel starts on
            # piece 0 (w + first 512 x cols, both c-tiles)
            W0 = XO + PIECE
            for ct in range(2):
                nc.sync.dma_start(out=blob_sb[:, ct, 0:W0],
                                  in_=blob256_d[ct * 128:(ct + 1) * 128, 0:W0])
            # identity/e4/ones right behind piece 0: the v-transposes need
            # the identity long before the projection needs woutP/bias
            nc.sync.dma_start(out=b128_sb[:, 2 * C:],
                              in_=blob128_d[:, 2 * C:])
            for xh in range(1, NJ // PIECE):
                lo = XO + xh * PIECE
                for ct in range(2):
                    nc.sync.dma_start(out=blob_sb[:, ct, lo:lo + PIECE],
                                      in_=blob256_d[ct * 128:(ct + 1) * 128,
                                                    lo:lo + PIECE])
            nc.sync.dma_start(out=b128_sb[:, 0:2 * C], in_=blob128_d[:, 0:2 * C])

            # trigger the ScalarE exp table load early
            warm = singles.tile([1, 1], F32)
            nc.vector.memset(warm, 0.0)
            nc.vector.memset(bm3_sb, -3.0)
            nc.scalar.activation(warm, warm, mybir.ActivationFunctionType.Exp)

            # big memsets go to GPSIMD: DVE must stay free for the phase-1
            # PSUM->SBUF copies that gate the first exp (GPSIMD is idle
            # until its first pow block at chunk 0 / jt 6)
            nc.gpsimd.memset(ebase_sb[:, :].bitcast(F32), float(np.exp(SCALE)))
            dstg0 = dsp.tile([128, CHUNK], F32, tag="dstg")
            nc.gpsimd.memset(dstg0, 1.0)
            nc.gpsimd.memset(vT_sb[:, :, :, :, DH], 1.0)   # ones column


            # ---- phase-1 projection helpers ----
            # [128, 512] PSUM sub-tiles through the pav A1/B1 tags (freed
            # fast by their DVE copy, no interaction with the sim tags that
            # pace ScalarE).
            p1_tag = [0]

            def p1_tile():
                # phase-1 tiles rotate through the sim tags (PSUM is fully
                # claimed by sim s0-s3 + av0-av3); each insert displaces one
                # sim allocation briefly, absorbed by the other heads' blocks
                t = psim.tile([128, CHUNK], F32, tag=f"s{p1_tag[0] % 4}",
                              name=f"p1_{p1_tag[0]}")
                p1_tag[0] += 1
                return t

            def emit_qkv_sub(kind, s, on_act=False):
                # kind: 0=q,1=k,2=v ; s: 512-col sub-tile index
                ps = p1_tile()
                for ct in range(2):
                    nc.tensor.matmul(
                        ps,
                        lhsT=w_sb[:, ct, kind * 128:(kind + 1) * 128],
                        rhs=x_sb[:, ct, s * PIECE:(s + 1) * PIECE],
                        start=(ct == 0), stop=(ct == 1),
                    )
                dst = (q_sb, k_sb, v_sb)[kind]
                cp = nc.scalar.copy if on_act else nc.vector.tensor_copy
                cp(dst[:, s * PIECE:(s + 1) * PIECE], ps)

            def emit_vt(jtt):
                # transpose v block jtt: [128 vc, 128 j] -> [128 j, 128 vc],
                # then scatter (head, dim) into the DoubleRow slab layout
                ps = p1_tile()
                nc.tensor.transpose(ps[:, 0:128].bitcast(F32R),
                                    v_sb[:, jtt * JT:(jtt + 1) * JT], id_sb)
                nc.vector.tensor_copy(vT_sb[:, jtt // 2, jtt % 2, :, 0:DH],
                                      ps[:, 0:128])

            # pre-loop: enough to start chunk 0 (q/k/v over pieces 0-1,
            # vT 0-7); everything else is spread thinly over chunk 0's
            # steps, each item emitted just ahead of its first consumer so
            # a DMA-gated item never head-of-line-blocks the sim stream
            # ScalarE is idle before the first exp: let it land the q/k
            # copies the first sims are waiting on
            emit_qkv_sub(0, 0, on_act=True)
            emit_qkv_sub(1, 0, on_act=True)
            emit_qkv_sub(1, 1)
            emit_qkv_sub(2, 0)
            for t in range(4):
                emit_vt(t)

            items = [
                [(0, 2, 1), (1, 4), (1, 5)],   # jt0: v j512:1024, t4-5
                [(1, 6), (1, 7)],
                [(0, 1, 2)],                   # k j1024:1536
                [(0, 2, 2)],
                [(1, 8), (1, 9)],
                [(1, 10), (1, 11)],
                [(0, 1, 3)],
                [(0, 2, 3)],
                [(1, 12), (1, 13)],
                [(1, 14), (1, 15)],
                [(0, 1, 4)],
                [(0, 2, 4)],
                [(1, 16), (1, 17)],
                [(1, 18), (1, 19)],
                [(0, 1, 5)],
                [(0, 2, 5)],
                [(1, 20), (1, 21)],
                [(1, 22), (1, 23)],
                [(0, 1, 6)],
                [(0, 2, 6)],
                [(1, 24), (1, 25)],
                [(1, 26), (1, 27)],
                [(0, 1, 7)],
                [(0, 2, 7)],
                [(1, 28), (1, 29)],
                [(1, 30), (1, 31)],
                [(0, 0, 1)],                   # q i512:1024
                [(0, 0, 2)],
                [(0, 0, 3)],
            ]

            def emit_items(step):
                if step < len(items):
                    for it in items[step]:
                        if it[0] == 0:
                            emit_qkv_sub(it[1], it[2], on_act=True)
                        else:
                            emit_vt(it[1])

            def emit_proj(co, it, tag, ot4=None):
                io = co + it * 128
                pj = psim.tile([128, CHUNK], F32, tag=tag,
                               name=f"pj_{co}_{it}") if tag.startswith("s") \
                    else pav.tile([128, CHUNK], F32, tag=tag,
                                  name=f"pj_{co}_{it}")
                pjv = pj[:, 0:C]
                nc.tensor.matmul(pjv, lhsT=h_sb[:, io:io + 128],
                                 rhs=woutP_sb, start=True, stop=False)
                # bias via ones-row outer product accumulated on the PE
                nc.tensor.matmul(pjv, lhsT=ones1_sb,
                                 rhs=bias_sb[0:1, :], start=False, stop=True)
                ot = outp.tile([128, C], F32, tag="out")
                if ot4 is None:
                    nc.vector.tensor_copy(ot, pjv)
                else:
                    nc.scalar.copy(ot, pjv)   # ACT is idle at the tail
                nc.sync.dma_start(out=out_d[io:io + 128, :], in_=ot)

            # ---- main attention loop ----
            for c, (co, W) in enumerate(CHUNKS):
                avh = []
                for h in range(HEADS):
                    av_t = pav.tile([128, CHUNK], F32, tag=f"av{h}",
                                    name=f"av{h}_c{c}")
                    avh.append(av_t)

                def emit_av(ex, jtp):
                    # fp8 DoubleRow: contraction = 128 j partitions x 2
                    # j-subtile slabs; out = all 128 partitions of av (rows
                    # 0:32 = out dims, 32 = denominator, 33:127 junk)
                    for h in range(HEADS):
                        nc.tensor.matmul(
                            avh[h][:, 0:W],
                            lhsT=vT_sb[:, jtp, :, h, :],
                            rhs=ex[:, :, h, 0:W],
                            start=(jtp == 0), stop=(jtp == NJT // 2 - 1),
                            perf_mode=mybir.MatmulPerfMode.DoubleRow,
                        )

                pending = None
                for jtp in range(NJT // 2):
                    ex = expp.tile([128, 2, HEADS, CHUNK], FP8, tag="ex")
                    for r in range(2):
                        jt = 2 * jtp + r
                        ph = _pool_heads(c, jt)
                        horder = [h for h in range(HEADS) if h not in ph] + \
                            [h for h in range(HEADS) if h in ph]
                        for h in horder:
                            sim = psim.tile([128, CHUNK], F32, tag=f"s{h}")
                            nc.tensor.matmul(
                                sim[:, 0:W],
                                lhsT=k_sb[h * DH:(h + 1) * DH,
                                          jt * JT:(jt + 1) * JT],
                                rhs=q_sb[h * DH:(h + 1) * DH, co:co + W],
                                start=True, stop=True,
                                tile_position=(h * DH, 0),
                            )
                            exs = ex[:, r, h, 0:W]
                            if h in ph:
                                pst = pstp.tile([128, CHUNK], F32, tag="pst")
                                # shift by 3/SCALE so exp = e^(scale*sim - 3)
                                # fits e4m3 (max scaled sim ~6.8 for N(0,1)
                                # activations; e^3.8 = 45 << 448)
                                nc.vector.tensor_scalar_sub(
                                    pst[:, 0:W], sim[:, 0:W], 3.0 / SCALE)
                                nc.gpsimd.tensor_tensor(
                                    out=exs, in0=ebase_sb[:, 0:W],
                                    in1=pst[:, 0:W],
                                    op=mybir.AluOpType.pow)
                            else:
                                nc.scalar.activation(
                                    exs, sim[:, 0:W],
                                    mybir.ActivationFunctionType.Exp,
                                    scale=SCALE, bias=bm3_sb)
                        if c == 0:
                            emit_items(jt)
                        if r == 0 and pending is not None:
                            emit_av(*pending)
                            pending = None
                    if pending is not None:
                        emit_av(*pending)
                    pending = (ex, jtp)
                emit_av(*pending)

                # ---- epilogue: softmax denominators ----
                # den rows live at psum partitions 32 & 96 of each av tile;
                # stage both rows of a tile with one stride-64 partition
                # copy, reciprocal both staged rows in one strided call,
                # ship all 4 rows to DRAM in one DMA, then 4 broadcast DMAs.
                # stage the 4 den rows (psum partitions 32/96 of avA/avB)
                # to partitions 0/32/64/96, h-major (engine partition bases
                # must be 32-aligned). Partition OFFSET shifts are legal for
                # a psum source. One full-partition reciprocal (cost is
                # free-size-bound); unused rows hold 1.0 (preset) so the e4
                # broadcast matmul contracts finite values against zeros.
                dstg = dsp.tile([128, CHUNK], F32, tag="dstg")
                rcps = dsp.tile([128, CHUNK], F32, tag="rcps")
                # ACT helps only in the last chunk (its exp stream is done);
                # mid-chunk epilogues must not block ACT's in-order queue
                cp2 = nc.scalar.copy
                nc.vector.tensor_copy(dstg[0:1, 0:W], avh[0][DH:DH + 1, 0:W])
                cp2(dstg[DH:DH + 1, 0:W], avh[1][DH:DH + 1, 0:W])
                nc.vector.tensor_copy(dstg[64:65, 0:W], avh[2][DH:DH + 1, 0:W])
                cp2(dstg[96:97, 0:W], avh[3][DH:DH + 1, 0:W])
                with nc.allow_low_precision("bf16 1/den feeds a bf16 "
                                            "broadcast matmul; 0.4% den "
                                            "error is well inside the gate"):
                    nc.vector.reciprocal(out=rcps[:, 0:W], in_=dstg[:, 0:W])
                # partition-broadcast 1/den_h to rows h*32..h*32+31 via a
                # K=128 ones-pattern matmul (PE; e4 row 32h carries head h),
                # then copy PSUM -> SBUF
                rbP = psim.tile([128, CHUNK], F32, tag="s0")
                nc.tensor.matmul(rbP[:, 0:W], lhsT=e4_sb.bitcast(F32),
                                 rhs=rcps[:, 0:W],
                                 start=True, stop=True)
                rbC = rbp.tile([128, CHUNK], F32, tag="rb")
                nc.vector.tensor_copy(rbC[:, 0:W], rbP[:, 0:W])
                # normalize into h-major rows (psum in0 + sbuf in1 may sit at
                # different base partitions; verifier only requires equality
                # for SB+SB operand pairs)
                nits = range(W // 128) if c == NCH - 1 else (None,)
                ot4 = c == NCH - 1 or None
                for nit in nits:
                    fs = slice(0, W) if nit is None else slice(nit * 128,
                                                               (nit + 1) * 128)
                    for h in range(HEADS):
                        nc.vector.tensor_tensor(
                            out=h_sb[h * DH:(h + 1) * DH,
                                     co + fs.start:co + fs.stop],
                            in0=avh[h][0:DH, fs],
                            in1=rbC[h * DH:(h + 1) * DH, fs],
                            op=mybir.AluOpType.mult,
                        )
                    if nit is not None:
                        # sim tags are idle at the tail; av tags stay locked
                        # until the last norm reads them
                        emit_proj(co, nit, f"s{1 + nit % 3}", ot4=ot4)
                if c < NCH - 1:
                    for it in range(W // 128):
                        emit_proj(co, it, f"av{it}")

    nc.compile()
    return nc


_NC = None


def _get_nc():
    global _NC
    if _NC is None:
        _NC = build_kernel()
    return _NC


def make_in_maps(x, w_qkv, w_out, b_out):
    x = np.ascontiguousarray(np.asarray(x, dtype=np.float32))
    w_qkv = np.asarray(w_qkv, dtype=np.float32)
    w_out = np.asarray(w_out, dtype=np.float32)
    b_out = np.asarray(b_out, dtype=np.float32)

    wqkvT = w_qkv.T                                       # [256, 384]
    # h_sb is head-major (rows h*32..h*32+31 = head h dims), so the
    # projection weight is just w_out transposed
    woutP = np.ascontiguousarray(w_out.T)                 # [128 hidden, 256]
    e4 = np.zeros((128, 128), np.float32)
    for h in range(4):
        e4[h * 32, h * 32:(h + 1) * 32] = 1.0
    ones1 = np.zeros((128, 128), np.float32)
    ones1[0, :] = 1.0
    blob128 = np.ascontiguousarray(
        np.concatenate([woutP,
                        np.broadcast_to(b_out[None, :], (128, C)),
                        np.eye(128, dtype=np.float32), e4, ones1], axis=1))

    in_maps = []
    for core in range(8):
        b, qh = divmod(core, 2)
        xb = x[b].reshape(C, NJ)
        # rotate keys so this core's queries are columns 0:NI
        xrot = np.concatenate([xb[:, qh * NI:], xb[:, :qh * NI]], axis=1)
        blob256 = np.ascontiguousarray(np.concatenate([wqkvT, xrot], axis=1))
        in_maps.append({"blob256": blob256, "blob128": blob128})
    return in_maps


def run_spmd(x, w_qkv, w_out, b_out, **kw):
    nc = _get_nc()
    in_maps = make_in_maps(x, w_qkv, w_out, b_out)
    return run_bass_kernel_spmd(nc, in_maps, core_ids=list(range(8)), **kw)


def assemble(results):
    out = np.empty((4, C, NJ), np.float32)
    for core in range(8):
        b, qh = divmod(core, 2)
        out[b, :, qh * NI:(qh + 1) * NI] = results[core]["out_t"].T
    return out.reshape(4, C, 64, 64)


def kernel(x, w_qkv, w_out, b_out):
    res = run_spmd(x, w_qkv, w_out, b_out)
    return assemble(res.results)
